# revision 2
# baseline (speedup 1.0000x reference)
"""Trainium2 Bass kernel for nn_DenseAtt: att[i,j] = sigmoid(x[i]@w1 + x[j]@w2 + b).

Sharding: rows of the (N, N) output split across 8 NeuronCores (1250 rows
each, as 10 groups of 125).  Per core:
  1. load its (F, 1250) x^T slab; compute its 1250 elements of s2 = x@w2
     on the PE and AllGather them (40KB) across the 8 cores,
  2. compute s1 = x_slab@w1 + b as a per-partition column [125, 10],
  3. replicate the gathered s2 row into a [125, N] bf16 SBUF tile with a
     single broadcast cast-DMA (partition-stride-0 DRAM read, gpsimd),
  4. produce output rows: ACT streams sigmoid(s2 + s1[p]) for 8 groups
     (exact LUT); DVE produces the last 2 groups with a clamped minimax
     quintic in bf16 (max err ~1.4e-2 < 2e-2 tolerance) so production
     exceeds a single engine's rate,
  5. write the 50MB slab with large DMAs round-robined over all three
     DMA-capable queues (sync/scalar HWDGE + gpsimd SWDGE), which run
     concurrently (~260-280 GB/s each).
"""

import math

import numpy as np

import concourse.bacc as bacc
import concourse.tile as tile
from concourse import mybir
from concourse.bass_utils import run_bass_kernel_spmd

N = 10000
F = 256
NCORES = 8
RPC = N // NCORES  # rows per core = 1250
P = 128
G = 125  # rows per output group
NG = RPC // G  # 10 groups
CJ = 512  # PSUM f32 bank chunk
CH = 2000  # DVE column chunk
NDVE = 2  # groups produced by DVE (rest on ACT)

# minimax clamped quintic for sigmoid on [-6.9, 6.9] (max err 9.5e-3):
# sigmoid(x) ~= clip(0.5 + x*(Q1 + t*(Q3 + Q5*t)), 0, 1), t = x*x
Q1 = 0.23343955
Q3 = -0.01114885
Q5 = 0.00024648

F32 = mybir.dt.float32
BF16 = mybir.dt.bfloat16

RDIM = {
    "all": 1,
    "coll": 2,
    "dma3q": 3,
    "dma2q": 4,
    "dma1q": 5,
    "dmabig3": 6,
    "act10k": 7,
    "dve": 8,
    "repl": 9,
    "prod": 10,
}


def build_bass(reps=1, timing=False, rep_scope="all"):
    """Per-core SPMD program.  Inputs (per core):
    xts (F, RPC) : x^T slab of this core's rows (f-major)
    wc  (F, 2)   : [w1 | w2] as columns
    bb  (P, 1)   : bias replicated per partition
    out (RPC, N) : this core's output slab

    reps/timing/rep_scope: differential-timing variants (see test.py/probe.py).
    """
    nc = bacc.Bacc("TRN2", target_bir_lowering=False, debug=False, num_devices=NCORES)
    xts_d = nc.declare_dram_parameter("xts", [F, RPC], F32, isOutput=False)
    wc_d = nc.declare_dram_parameter("wc", [F, 2], F32, isOutput=False)
    bb_d = nc.declare_dram_parameter("bb", [P, 1], F32, isOutput=False)
    rtag = None
    if reps > 1 or timing:
        rdim = RDIM[rep_scope]
        rtag = nc.declare_dram_parameter("rtag", [rdim, reps], F32, isOutput=False)
    if timing:
        out = nc.dram_tensor("out_scratch", [RPC, N], F32)
        ok = nc.declare_dram_parameter("ok", [1, 4], F32, isOutput=True)
    else:
        out = nc.declare_dram_parameter("out", [RPC, N], F32, isOutput=True)
        ok = None

    nact = NG - NDVE  # ACT-produced groups
    QUEUES = ["sync", "scalar", "gpsimd"]

    with tile.TileContext(nc) as tc:
        with (
            tc.tile_pool(name="consts", bufs=1) as consts,
            tc.tile_pool(name="xsp", bufs=1) as xsp,
            tc.tile_pool(name="s2sp", bufs=1) as s2sp,
            tc.tile_pool(name="s2rep", bufs=2) as s2rep_pool,
            tc.tile_pool(name="tmp", bufs=1) as tmp,
            tc.tile_pool(name="stage", bufs=3) as stagep,
            tc.tile_pool(name="dstage", bufs=2) as dstagep,
            tc.tile_pool(name="psum2", bufs=2, space="PSUM") as psum2,
            tc.tile_pool(name="psum1", bufs=2, space="PSUM") as psum1,
            tc.tile_pool(name="dram", bufs=1, space="DRAM") as dram,
        ):

            def eng(i):
                return getattr(nc, QUEUES[i % 3])

            if rtag is not None:
                rtag_sb = consts.tile(list(rtag.shape), F32, tag="rtag")
                nc.scalar.dma_start(out=rtag_sb, in_=rtag[:, :])

            def load_consts():
                wc_sb = consts.tile([P, 2, 2], F32, tag="wc")
                nc.scalar.dma_start(out=wc_sb[:, 0, :], in_=wc_d[0:P, :])
                nc.scalar.dma_start(out=wc_sb[:, 1, :], in_=wc_d[P : 2 * P, :])
                b_sb = consts.tile([P, 1], F32, tag="bb")
                nc.scalar.dma_start(out=b_sb, in_=bb_d[:, :])
                return wc_sb, b_sb

            def load_xts():
                xts_sb = xsp.tile([P, 2, RPC], F32, tag="xts")
                h = RPC // 2
                nc.sync.dma_start(out=xts_sb[:, 0, 0:h], in_=xts_d[0:P, 0:h])
                nc.sync.dma_start(out=xts_sb[:, 0, h:RPC], in_=xts_d[0:P, h:RPC])
                nc.scalar.dma_start(out=xts_sb[:, 1, 0:h], in_=xts_d[P : 2 * P, 0:h])
                nc.scalar.dma_start(
                    out=xts_sb[:, 1, h:RPC], in_=xts_d[P : 2 * P, h:RPC]
                )
                return xts_sb

            def compute_s2s(xts_sb, wc_sb):
                # own 1250 elements of s2 = x @ w2 as a [1, RPC] row
                s2s_sb = s2sp.tile([1, RPC], F32, tag="s2s")
                for sj in range(0, RPC, CJ):
                    cw = min(CJ, RPC - sj)
                    pss = psum2.tile([1, CJ], F32, tag="pss")
                    nc.tensor.matmul(
                        out=pss[0:1, :cw],
                        lhsT=wc_sb[:, 0, 1:2],
                        rhs=xts_sb[:, 0, sj : sj + cw],
                        start=True,
                        stop=False,
                    )
                    nc.tensor.matmul(
                        out=pss[0:1, :cw],
                        lhsT=wc_sb[:, 1, 1:2],
                        rhs=xts_sb[:, 1, sj : sj + cw],
                        start=False,
                        stop=True,
                    )
                    nc.vector.tensor_copy(
                        out=s2s_sb[0:1, sj : sj + cw], in_=pss[0:1, :cw]
                    )
                return s2s_sb

            def allgather(s2s_sb):
                in_b = dram.tile([1, RPC], F32, tag="in_b")
                out_b = dram.tile([1, N], F32, tag="out_b")
                nc.scalar.dma_start(out=in_b[:, :], in_=s2s_sb[:, :])
                nc.gpsimd.collective_compute(
                    "AllGather",
                    mybir.AluOpType.bypass,
                    replica_groups=[list(range(NCORES))],
                    ins=[in_b[:, :]],
                    outs=[out_b[:, :]],
                )
                return out_b

            def compute_s1(xts_sb, wc_sb, b_sb):
                # s1[125g+p] + b -> s1_sb[p, g]
                s1_sb = consts.tile([G, NG], F32, tag="s1")
                for g in range(NG):
                    r0 = g * G
                    ps1 = psum1.tile([G, 8], F32, tag="ps1")
                    nc.tensor.matmul(
                        out=ps1[:G, 0:1],
                        lhsT=xts_sb[:, 0, r0 : r0 + G],
                        rhs=wc_sb[:, 0, 0:1],
                        start=True,
                        stop=False,
                    )
                    nc.tensor.matmul(
                        out=ps1[:G, 0:1],
                        lhsT=xts_sb[:, 1, r0 : r0 + G],
                        rhs=wc_sb[:, 1, 0:1],
                        start=False,
                        stop=True,
                    )
                    nc.vector.tensor_scalar_add(
                        out=s1_sb[:G, g : g + 1], in0=ps1[:G, 0:1], scalar1=b_sb[:G, :]
                    )
                return s1_sb

            def replicate(out_b):
                # broadcast the gathered s2 row into all 125 partitions with a
                # single stride-0 cast-DMA (f32 DRAM -> bf16 SBUF, SWDGE)
                s2_rep = s2rep_pool.tile([G, N], BF16, tag="s2rep")
                nc.gpsimd.dma_start(
                    out=s2_rep[:, :], in_=out_b[0:1, :].broadcast_to((G, N))
                )
                return s2_rep

            def act_group(s2_rep, s1_sb, g, o_t, c0=0, c1=N):
                nc.scalar.activation(
                    out=o_t[:G, c0:c1],
                    in_=s2_rep[:G, c0:c1],
                    func=mybir.ActivationFunctionType.Sigmoid,
                    bias=s1_sb[:G, g : g + 1],
                    scale=1.0,
                )

            def dve_chunk(s2_rep, s1_sb, g, d_t, c0, cw):
                x_t = tmp.tile([G, CH], BF16, tag="xt")
                t_t = tmp.tile([G, CH], BF16, tag="tt")
                v_t = tmp.tile([G, CH], BF16, tag="vt")
                x, t, v = x_t[:G, :cw], t_t[:G, :cw], v_t[:G, :cw]
                nc.vector.tensor_scalar_add(
                    out=x, in0=s2_rep[:G, c0 : c0 + cw], scalar1=s1_sb[:G, g : g + 1]
                )
                nc.vector.tensor_tensor(out=t, in0=x, in1=x, op=mybir.AluOpType.mult)
                nc.vector.tensor_scalar(
                    out=v, in0=t, scalar1=Q5, scalar2=Q3,
                    op0=mybir.AluOpType.mult, op1=mybir.AluOpType.add,
                )
                nc.vector.tensor_tensor(out=v, in0=v, in1=t, op=mybir.AluOpType.mult)
                nc.vector.tensor_scalar_add(out=v, in0=v, scalar1=Q1)
                nc.vector.tensor_tensor(out=x, in0=x, in1=v, op=mybir.AluOpType.mult)
                nc.vector.tensor_scalar(
                    out=x, in0=x, scalar1=0.5, scalar2=0.0,
                    op0=mybir.AluOpType.add, op1=mybir.AluOpType.max,
                )
                nc.vector.tensor_scalar_min(out=d_t[:G, :cw], in0=x, scalar1=1.0)

            # ---------------- probe scopes ----------------
            if rep_scope in ("dma3q", "dma2q", "dma1q", "dmabig3"):
                tiles = []
                for i in range(3):
                    o_t = stagep.tile([G, N], F32, tag="o_t", name=f"st{i}")
                    nc.vector.memset(o_t, 0.25)
                    tiles.append(o_t)
                nq = {"dma3q": 3, "dma2q": 2, "dma1q": 1, "dmabig3": 3}[rep_scope]
                half = rep_scope != "dmabig3"
                di = 0
                for _r in range(reps):
                    for g in range(NG):
                        r0 = g * G
                        src = tiles[g % 3]
                        if half:
                            for c0 in (0, N // 2):
                                eng(di % nq).dma_start(
                                    out=out[r0 : r0 + G, c0 : c0 + N // 2],
                                    in_=src[:, c0 : c0 + N // 2],
                                )
                                di += 1
                        else:
                            eng(di % nq).dma_start(
                                out=out[r0 : r0 + G, :], in_=src[:, :]
                            )
                            di += 1
            elif rep_scope == "coll":
                wc_sb, b_sb = load_consts()
                xts_sb = load_xts()
                s2s_sb = compute_s2s(xts_sb, wc_sb)
                for _r in range(reps):
                    out_b = allgather(s2s_sb)
                s2_rep = replicate(out_b)
                o_t = stagep.tile([G, N], F32, tag="o_t")
                s1_sb = compute_s1(xts_sb, wc_sb, b_sb)
                act_group(s2_rep, s1_sb, 0, o_t)
                nc.sync.dma_start(out=out[0:G, :], in_=o_t[:, :])
            elif rep_scope == "repl":
                wc_sb, b_sb = load_consts()
                xts_sb = load_xts()
                s2s_sb = compute_s2s(xts_sb, wc_sb)
                out_b = allgather(s2s_sb)
                s1_sb = compute_s1(xts_sb, wc_sb, b_sb)
                for _r in range(reps):
                    s2_rep = replicate(out_b)
                o_t = stagep.tile([G, N], F32, tag="o_t")
                act_group(s2_rep, s1_sb, 0, o_t)
                nc.sync.dma_start(out=out[0:G, :], in_=o_t[:, :])
            elif rep_scope in ("act10k", "dve", "prod", "all", "main"):
                per_rep = rep_scope in ("all", "main")
                wc_sb, b_sb = load_consts()
                if not per_rep:
                    xts_sb = load_xts()
                    s2s_sb = compute_s2s(xts_sb, wc_sb)
                    out_b = allgather(s2s_sb)
                    s1_sb = compute_s1(xts_sb, wc_sb, b_sb)
                    s2_rep = replicate(out_b)
                for _r in range(reps):
                    if per_rep:
                        xts_sb = load_xts()
                        s2s_sb = compute_s2s(xts_sb, wc_sb)
                        out_b = allgather(s2s_sb)
                        s1_sb = compute_s1(xts_sb, wc_sb, b_sb)
                        s2_rep = replicate(out_b)
                    if rep_scope == "act10k":
                        for g in range(NG):
                            o_t = stagep.tile([G, N], F32, tag="o_t")
                            act_group(s2_rep, s1_sb, g, o_t)
                        continue
                    if rep_scope == "dve":
                        for c0 in range(0, N, CH):
                            cw = min(CH, N - c0)
                            d_t = dstagep.tile([G, CH], F32, tag="d_t")
                            dve_chunk(s2_rep, s1_sb, NG - 1, d_t, c0, cw)
                        continue
                    # act10k-style production + (for all) DMAs
                    do_dma = rep_scope in ("all", "main")
                    di = 0
                    for g in range(nact):
                        r0 = g * G
                        o_t = stagep.tile([G, N], F32, tag="o_t")
                        act_group(s2_rep, s1_sb, g, o_t)
                        if do_dma:
                            for c0 in (0, N // 2):
                                eng(di).dma_start(
                                    out=out[r0 : r0 + G, c0 : c0 + N // 2],
                                    in_=o_t[:, c0 : c0 + N // 2],
                                )
                                di += 1
                    for g in range(nact, NG):
                        r0 = g * G
                        for c0 in range(0, N, CH):
                            cw = min(CH, N - c0)
                            d_t = dstagep.tile([G, CH], F32, tag="d_t")
                            dve_chunk(s2_rep, s1_sb, g, d_t, c0, cw)
                            if do_dma:
                                eng(di).dma_start(
                                    out=out[r0 : r0 + G, c0 : c0 + cw],
                                    in_=d_t[:G, :cw],
                                )
                                di += 1

            if ok is not None:
                okt = consts.tile([1, 4], F32, tag="okt")
                nc.sync.dma_start(out=okt, in_=out[0:1, 0:4])
                nc.sync.dma_start(out=ok[:, :], in_=okt)
    nc.compile()
    return nc


_NC = {}


def _get_nc(reps=1, timing=False, rep_scope="all"):
    key = (reps, timing, rep_scope)
    if key not in _NC:
        _NC[key] = build_bass(reps=reps, timing=timing, rep_scope=rep_scope)
    return _NC[key]


def make_in_maps(x, w, b):
    xT = np.ascontiguousarray(x.T)  # (F, N)
    wc = np.ascontiguousarray(np.stack([w[0, :F], w[0, F:]], axis=1))  # (F, 2)
    bb = np.full((P, 1), np.float32(b[0]), dtype=np.float32)
    in_maps = []
    for c in range(NCORES):
        xts = np.ascontiguousarray(xT[:, c * RPC : (c + 1) * RPC])
        in_maps.append({"xts": xts, "wc": wc, "bb": bb})
    return in_maps


def kernel(x, adj, w, b):
    x = np.asarray(x, dtype=np.float32)
    w = np.asarray(w, dtype=np.float32)
    b = np.asarray(b, dtype=np.float32)
    nc = _get_nc()
    in_maps = make_in_maps(x, w, b)
    res = run_bass_kernel_spmd(nc, in_maps, list(range(NCORES)))
    return np.concatenate([res.results[c]["out"] for c in range(NCORES)], axis=0)


# revision 8
# speedup vs baseline: 1.2798x; 1.2798x over previous
"""Trainium2 Bass kernel for nn_DenseAtt: att[i,j] = sigmoid(x[i]@w1 + x[j]@w2 + b).

Sharding: rows of the (N, N) output split across 8 NeuronCores (1250 rows
each, as 9 groups of 128 rows + one of 98).  Per core:
  1. load its (F, 1250) x^T slab; compute its 1250 elements of s2 = x@w2
     on the PE and AllGather them (40KB) across the 8 cores,
  2. compute s1 = x_slab@w1 + b as per-partition columns [128, 10],
  3. replicate the gathered s2 row into a [128, N] bf16 SBUF tile with a
     single broadcast cast-DMA (partition-stride-0 DRAM read, SWDGE),
  4. ACT streams sigmoid(s2 + s1[p]) per group (exact LUT, one full-width
     instruction per group; ~1 elem/cycle/lane @1.4GHz ~= 670 GB/s),
  5. write the 50MB slab with 5MB DMAs spread over all three DMA-capable
     queues (sync/scalar HWDGE + gpsimd SWDGE), which drain concurrently.
128-partition tiles matter: sub-128-partition DMAs run ~2.4x slower.
Optionally (NDVE>0) DVE produces the last full group(s) with a clamped
minimax quintic sigmoid in bf16 to offload ACT.
"""

import math

import numpy as np

import concourse.bacc as bacc
import concourse.tile as tile
from concourse import mybir
from concourse.bass_utils import run_bass_kernel_spmd

N = 10000
F = 256
NCORES = 8
RPC = N // NCORES  # rows per core = 1250
P = 128
CJ = 512  # PSUM f32 bank chunk
CH = 2000  # DVE column chunk
NDVE = 0  # full groups produced by DVE instead of ACT

# row groups per core: 9 x 128 + 98
GROUPS = [(g * P, P) for g in range(9)] + [(9 * P, RPC - 9 * P)]
NG = len(GROUPS)

# minimax clamped quintic for sigmoid on [-6.9, 6.9] (max err 9.5e-3):
# sigmoid(x) ~= clip(0.5 + x*(Q1 + t*(Q3 + Q5*t)), 0, 1), t = x*x
Q1 = 0.23343955
Q3 = -0.01114885
Q5 = 0.00024648

F32 = mybir.dt.float32
BF16 = mybir.dt.bfloat16

# output-DMA queue plan: queue index (0=sync,1=scalar,2=gpsimd) per DMA op,
# cycled.  Tail group issues 2 column-half DMAs.
QPLAN = [0, 1, 2]

RDIM = {
    "all": 1,
    "coll": 2,
    "repl": 3,
    "act10k": 4,
    "dve": 5,
    "prod": 6,
    "dmaA": 11,
    "dmaD": 14,
    "dmaE": 15,
    "dmaF": 16,
    "dmaG": 17,
    "dmaH": 18,
}


def build_bass(reps=1, timing=False, rep_scope="all"):
    """Per-core SPMD program.  Inputs (per core):
    xts (F, RPC) : x^T slab of this core's rows (f-major)
    wc  (F, 2)   : [w1 | w2] as columns
    bb  (P, 1)   : bias replicated per partition
    out (RPC, N) : this core's output slab

    reps/timing/rep_scope: differential-timing variants (see test.py/probe.py).
    """
    nc = bacc.Bacc("TRN2", target_bir_lowering=False, debug=False, num_devices=NCORES)
    xts_d = nc.declare_dram_parameter("xts", [F, RPC], F32, isOutput=False)
    wc_d = nc.declare_dram_parameter("wc", [F, 2], F32, isOutput=False)
    bb_d = nc.declare_dram_parameter("bb", [P, 1], F32, isOutput=False)
    rtag = None
    if reps > 1 or timing:
        rdim = RDIM[rep_scope]
        rtag = nc.declare_dram_parameter("rtag", [rdim, reps], F32, isOutput=False)
    if timing:
        out = nc.dram_tensor("out_scratch", [RPC, N], F32)
        ok = nc.declare_dram_parameter("ok", [1, 4], F32, isOutput=True)
    else:
        out = nc.declare_dram_parameter("out", [RPC, N], F32, isOutput=True)
        ok = None

    QUEUES = ["sync", "scalar", "gpsimd"]

    with tile.TileContext(nc) as tc:
        with (
            tc.tile_pool(name="consts", bufs=1) as consts,
            tc.tile_pool(name="xsp", bufs=1) as xsp,
            tc.tile_pool(name="s2sp", bufs=1) as s2sp,
            tc.tile_pool(name="s2rep", bufs=2) as s2rep_pool,
            tc.tile_pool(name="tmp", bufs=1) as tmp,
            tc.tile_pool(name="stage", bufs=3) as stagep,
            tc.tile_pool(name="dstage", bufs=2) as dstagep,
            tc.tile_pool(name="psum2", bufs=2, space="PSUM") as psum2,
            tc.tile_pool(name="psum1", bufs=2, space="PSUM") as psum1,
            tc.tile_pool(name="dram", bufs=1, space="DRAM") as dram,
        ):

            def qeng(i):
                return getattr(nc, QUEUES[QPLAN[i % len(QPLAN)]])

            if rtag is not None:
                rtag_sb = consts.tile(list(rtag.shape), F32, tag="rtag")
                nc.scalar.dma_start(out=rtag_sb, in_=rtag[:, :])

            def load_consts():
                wc_sb = consts.tile([P, 2, 2], F32, tag="wc")
                nc.scalar.dma_start(out=wc_sb[:, 0, :], in_=wc_d[0:P, :])
                nc.scalar.dma_start(out=wc_sb[:, 1, :], in_=wc_d[P : 2 * P, :])
                b_sb = consts.tile([P, 1], F32, tag="bb")
                nc.scalar.dma_start(out=b_sb, in_=bb_d[:, :])
                return wc_sb, b_sb

            def load_xts():
                xts_sb = xsp.tile([P, 2, RPC], F32, tag="xts")
                h = RPC // 2
                nc.sync.dma_start(out=xts_sb[:, 0, 0:h], in_=xts_d[0:P, 0:h])
                nc.sync.dma_start(out=xts_sb[:, 0, h:RPC], in_=xts_d[0:P, h:RPC])
                nc.scalar.dma_start(out=xts_sb[:, 1, 0:h], in_=xts_d[P : 2 * P, 0:h])
                nc.scalar.dma_start(
                    out=xts_sb[:, 1, h:RPC], in_=xts_d[P : 2 * P, h:RPC]
                )
                return xts_sb

            def compute_s2s(xts_sb, wc_sb):
                # own 1250 elements of s2 = x @ w2 as a [1, RPC] row
                s2s_sb = s2sp.tile([1, RPC], F32, tag="s2s")
                for sj in range(0, RPC, CJ):
                    cw = min(CJ, RPC - sj)
                    pss = psum2.tile([1, CJ], F32, tag="pss")
                    nc.tensor.matmul(
                        out=pss[0:1, :cw],
                        lhsT=wc_sb[:, 0, 1:2],
                        rhs=xts_sb[:, 0, sj : sj + cw],
                        start=True,
                        stop=False,
                    )
                    nc.tensor.matmul(
                        out=pss[0:1, :cw],
                        lhsT=wc_sb[:, 1, 1:2],
                        rhs=xts_sb[:, 1, sj : sj + cw],
                        start=False,
                        stop=True,
                    )
                    nc.vector.tensor_copy(
                        out=s2s_sb[0:1, sj : sj + cw], in_=pss[0:1, :cw]
                    )
                return s2s_sb

            def allgather(s2s_sb):
                in_b = dram.tile([1, RPC], F32, tag="in_b")
                out_b = dram.tile([1, N], F32, tag="out_b")
                nc.scalar.dma_start(out=in_b[:, :], in_=s2s_sb[:, :])
                nc.gpsimd.collective_compute(
                    "AllGather",
                    mybir.AluOpType.bypass,
                    replica_groups=[list(range(NCORES))],
                    ins=[in_b[:, :]],
                    outs=[out_b[:, :]],
                )
                return out_b

            def compute_s1(xts_sb, wc_sb, b_sb):
                # s1[r0+p] + b -> s1_sb[p, g]
                s1_sb = consts.tile([P, NG], F32, tag="s1")
                for g, (r0, rt) in enumerate(GROUPS):
                    ps1 = psum1.tile([P, 8], F32, tag="ps1")
                    nc.tensor.matmul(
                        out=ps1[:rt, 0:1],
                        lhsT=xts_sb[:, 0, r0 : r0 + rt],
                        rhs=wc_sb[:, 0, 0:1],
                        start=True,
                        stop=False,
                    )
                    nc.tensor.matmul(
                        out=ps1[:rt, 0:1],
                        lhsT=xts_sb[:, 1, r0 : r0 + rt],
                        rhs=wc_sb[:, 1, 0:1],
                        start=False,
                        stop=True,
                    )
                    nc.vector.tensor_scalar_add(
                        out=s1_sb[:rt, g : g + 1],
                        in0=ps1[:rt, 0:1],
                        scalar1=b_sb[:rt, :],
                    )
                return s1_sb

            def replicate(out_b):
                # broadcast the gathered s2 row into all 128 partitions with a
                # single stride-0 cast-DMA (f32 DRAM -> bf16 SBUF, SWDGE)
                s2_rep = s2rep_pool.tile([P, N], BF16, tag="s2rep")
                nc.gpsimd.dma_start(
                    out=s2_rep[:, :], in_=out_b[0:1, :].broadcast_to((P, N))
                )
                return s2_rep

            def act_group(s2_rep, s1_sb, g, rt, o_t):
                nc.scalar.activation(
                    out=o_t[:rt, :],
                    in_=s2_rep[:rt, :],
                    func=mybir.ActivationFunctionType.Sigmoid,
                    bias=s1_sb[:rt, g : g + 1],
                    scale=1.0,
                )

            def dve_chunk(s2_rep, s1_sb, g, rt, d_t, c0, cw):
                x_t = tmp.tile([P, CH], BF16, tag="xt")
                t_t = tmp.tile([P, CH], BF16, tag="tt")
                v_t = tmp.tile([P, CH], BF16, tag="vt")
                x, t, v = x_t[:rt, :cw], t_t[:rt, :cw], v_t[:rt, :cw]
                nc.vector.tensor_scalar_add(
                    out=x, in0=s2_rep[:rt, c0 : c0 + cw], scalar1=s1_sb[:rt, g : g + 1]
                )
                nc.vector.tensor_tensor(out=t, in0=x, in1=x, op=mybir.AluOpType.mult)
                nc.vector.tensor_scalar(
                    out=v, in0=t, scalar1=Q5, scalar2=Q3,
                    op0=mybir.AluOpType.mult, op1=mybir.AluOpType.add,
                )
                nc.vector.tensor_tensor(out=v, in0=v, in1=t, op=mybir.AluOpType.mult)
                nc.vector.tensor_scalar_add(out=v, in0=v, scalar1=Q1)
                nc.vector.tensor_tensor(out=x, in0=x, in1=v, op=mybir.AluOpType.mult)
                nc.vector.tensor_scalar(
                    out=x, in0=x, scalar1=0.5, scalar2=0.0,
                    op0=mybir.AluOpType.add, op1=mybir.AluOpType.max,
                )
                nc.vector.tensor_scalar_min(out=d_t[:rt, :cw], in0=x, scalar1=1.0)

            def out_group_dma(g, r0, rt, o_t, di):
                # full groups: one 5MB DMA; tail: split into 2 halves
                if rt == P:
                    qeng(di).dma_start(out=out[r0 : r0 + rt, :], in_=o_t[:rt, :])
                    return di + 1
                for c0 in (0, N // 2):
                    qeng(di).dma_start(
                        out=out[r0 : r0 + rt, c0 : c0 + N // 2],
                        in_=o_t[:rt, c0 : c0 + N // 2],
                    )
                    di += 1
                return di

            # ---------------- probe scopes ----------------
            if rep_scope in ("dmaA", "dmaD", "dmaE", "dmaF", "dmaG", "dmaH"):
                # big-DMA rate probes, [128, N] tiles, rows 128g:
                # A: round-robin sync/scalar/gpsimd   D: sync/scalar
                # E: gpsimd only   F: sync only   G: gpsimd/sync
                # H: gpsimd-heavy (g,s,g,a)
                qmap = {
                    "dmaA": [0, 1, 2],
                    "dmaD": [0, 1],
                    "dmaE": [2],
                    "dmaF": [0],
                    "dmaG": [2, 0],
                    "dmaH": [2, 0, 2, 1],
                }[rep_scope]
                tiles = []
                for i in range(3):
                    o_t = stagep.tile([P, N], F32, tag="o_t", name=f"st{i}")
                    nc.vector.memset(o_t, 0.25)
                    tiles.append(o_t)
                di = 0
                for _r in range(reps):
                    for g in range(9):
                        r0 = g * P
                        e = getattr(nc, QUEUES[qmap[di % len(qmap)]])
                        e.dma_start(out=out[r0 : r0 + P, :], in_=tiles[g % 3][:, :])
                        di += 1
            elif rep_scope == "coll":
                wc_sb, b_sb = load_consts()
                xts_sb = load_xts()
                s2s_sb = compute_s2s(xts_sb, wc_sb)
                for _r in range(reps):
                    out_b = allgather(s2s_sb)
                s2_rep = replicate(out_b)
                o_t = stagep.tile([P, N], F32, tag="o_t")
                s1_sb = compute_s1(xts_sb, wc_sb, b_sb)
                act_group(s2_rep, s1_sb, 0, P, o_t)
                nc.sync.dma_start(out=out[0:P, :], in_=o_t[:, :])
            elif rep_scope == "repl":
                wc_sb, b_sb = load_consts()
                xts_sb = load_xts()
                s2s_sb = compute_s2s(xts_sb, wc_sb)
                out_b = allgather(s2s_sb)
                s1_sb = compute_s1(xts_sb, wc_sb, b_sb)
                for _r in range(reps):
                    s2_rep = replicate(out_b)
                o_t = stagep.tile([P, N], F32, tag="o_t")
                act_group(s2_rep, s1_sb, 0, P, o_t)
                nc.sync.dma_start(out=out[0:P, :], in_=o_t[:, :])
            elif rep_scope in ("act10k", "dve", "prod", "all"):
                per_rep = rep_scope == "all"
                wc_sb, b_sb = load_consts()
                if not per_rep:
                    xts_sb = load_xts()
                    s2s_sb = compute_s2s(xts_sb, wc_sb)
                    out_b = allgather(s2s_sb)
                    s1_sb = compute_s1(xts_sb, wc_sb, b_sb)
                    s2_rep = replicate(out_b)
                for _r in range(reps):
                    if per_rep:
                        xts_sb = load_xts()
                        s2s_sb = compute_s2s(xts_sb, wc_sb)
                        out_b = allgather(s2s_sb)
                        s1_sb = compute_s1(xts_sb, wc_sb, b_sb)
                        s2_rep = replicate(out_b)
                    if rep_scope == "act10k":
                        for g, (r0, rt) in enumerate(GROUPS):
                            o_t = stagep.tile([P, N], F32, tag="o_t")
                            act_group(s2_rep, s1_sb, g, rt, o_t)
                        continue
                    if rep_scope == "dve":
                        for c0 in range(0, N, CH):
                            cw = min(CH, N - c0)
                            d_t = dstagep.tile([P, CH], F32, tag="d_t")
                            dve_chunk(s2_rep, s1_sb, 8, P, d_t, c0, cw)
                        continue
                    do_dma = rep_scope == "all"
                    di = 0
                    for g, (r0, rt) in enumerate(GROUPS):
                        # DVE takes the last NDVE full (128-row) groups
                        if NDVE > 0 and 9 - NDVE <= g < 9:
                            for c0 in range(0, N, CH):
                                cw = min(CH, N - c0)
                                d_t = dstagep.tile([P, CH], F32, tag="d_t")
                                dve_chunk(s2_rep, s1_sb, g, rt, d_t, c0, cw)
                                if do_dma:
                                    qeng(di).dma_start(
                                        out=out[r0 : r0 + rt, c0 : c0 + cw],
                                        in_=d_t[:rt, :cw],
                                    )
                                    di += 1
                        else:
                            o_t = stagep.tile([P, N], F32, tag="o_t")
                            act_group(s2_rep, s1_sb, g, rt, o_t)
                            if do_dma:
                                di = out_group_dma(g, r0, rt, o_t, di)

            if ok is not None:
                okt = consts.tile([1, 4], F32, tag="okt")
                nc.sync.dma_start(out=okt, in_=out[0:1, 0:4])
                nc.sync.dma_start(out=ok[:, :], in_=okt)
    nc.compile()
    return nc


_NC = {}


def _get_nc(reps=1, timing=False, rep_scope="all"):
    key = (reps, timing, rep_scope)
    if key not in _NC:
        _NC[key] = build_bass(reps=reps, timing=timing, rep_scope=rep_scope)
    return _NC[key]


def make_in_maps(x, w, b):
    xT = np.ascontiguousarray(x.T)  # (F, N)
    wc = np.ascontiguousarray(np.stack([w[0, :F], w[0, F:]], axis=1))  # (F, 2)
    bb = np.full((P, 1), np.float32(b[0]), dtype=np.float32)
    in_maps = []
    for c in range(NCORES):
        xts = np.ascontiguousarray(xT[:, c * RPC : (c + 1) * RPC])
        in_maps.append({"xts": xts, "wc": wc, "bb": bb})
    return in_maps


def kernel(x, adj, w, b):
    x = np.asarray(x, dtype=np.float32)
    w = np.asarray(w, dtype=np.float32)
    b = np.asarray(b, dtype=np.float32)
    nc = _get_nc()
    in_maps = make_in_maps(x, w, b)
    res = run_bass_kernel_spmd(nc, in_maps, list(range(NCORES)))
    return np.concatenate([res.results[c]["out"] for c in range(NCORES)], axis=0)


# revision 15
# speedup vs baseline: 2.4102x; 1.8833x over previous
"""Trainium2 Bass kernel for nn_DenseAtt: att[i,j] = sigmoid(x[i]@w1 + x[j]@w2 + b).

Sharding: rows of the (N, N) output split across 8 NeuronCores (1250 rows
each, as 10 groups of 128 rows; the last group overlaps the 9th by 30 rows
so every DMA keeps 128 partitions — sub-128-partition DMAs run ~2.4x
slower).  Per core:
  1. load its (F, 1250) x^T slab; compute its 1250 elements of s2 = x@w2
     on the PE (cast bf16) and AllGather them (20KB) across the 8 cores,
  2. compute s1 = x_slab@w1 + b as per-partition columns [128, 10],
  3. replicate the gathered s2 row into a [128, N] bf16 SBUF tile with
     K=1 ones-matmuls on the otherwise-idle PE (bf16 keeps SBUF small;
     its rounding adds <2e-3 error vs the 2e-2 tolerance),
  4. ACT streams sigmoid(s2 + s1[p]) per group (exact LUT, one full-width
     [128, 10000] instruction per group; ~1 elem/cycle/lane ~= 670 GB/s),
  5. write the slab with 5MB full-row-contiguous DMAs alternating over the
     two HWDGE queues (sync/scalar), which drain concurrently; a single
     queue or a 3-way mix with gpsimd both measured slower.
Optionally (NDVE>0) DVE produces the last full group(s) with a clamped
minimax quintic sigmoid in bf16 to offload ACT.
"""

import numpy as np

import concourse.bacc as bacc
import concourse.tile as tile
from concourse import mybir
from concourse.bass_utils import run_bass_kernel_spmd

N = 10000
F = 256
NCORES = 8
RPC = N // NCORES  # rows per core = 1250
P = 128
CJ = 512  # PSUM f32 bank chunk
CH = 2000  # DVE column chunk
NDVE = 0  # full groups produced by DVE instead of ACT

# row groups per core: 10 x 128, the last overlapping the 9th by 30 rows
# (rows 1122-1151 are written twice with identical values).  Sub-128-
# partition DMAs run ~2.4x slower, so a full overlapped group wins.
GROUPS = [(g * P, P) for g in range(9)] + [(RPC - P, P)]
NG = len(GROUPS)

# minimax clamped quintic for sigmoid on [-6.9, 6.9] (max err 9.5e-3):
# sigmoid(x) ~= clip(0.5 + x*(Q1 + t*(Q3 + Q5*t)), 0, 1), t = x*x
Q1 = 0.23343955
Q3 = -0.01114885
Q5 = 0.00024648

F32 = mybir.dt.float32
BF16 = mybir.dt.bfloat16

# output-DMA queue plan: queue index (0=sync,1=scalar,2=gpsimd) per DMA op,
# cycled.  Two concurrent queues saturate the HBM write path; mixing all
# three measured slower.  Overridable for experiments via env QPLAN="02".
import os as _os

QPLAN = [int(c) for c in _os.environ.get("QPLAN", "01")]

RDIM = {
    "all": 1,
    "coll": 2,
    "repl": 3,
    "act10k": 4,
    "dve": 5,
    "prod": 6,
    "dmaA": 11,
    "dmaD": 14,
    "dmaE": 15,
    "dmaF": 16,
    "dmaG": 17,
    "dmaH": 18,
}


def build_bass(reps=1, timing=False, rep_scope="all"):
    """Per-core SPMD program.  Inputs (per core):
    xts (F, RPC) : x^T slab of this core's rows (f-major)
    wc  (F, 2)   : [w1 | w2] as columns
    bb  (P, 1)   : bias replicated per partition
    out (RPC, N) : this core's output slab

    reps/timing/rep_scope: differential-timing variants (see test.py/probe.py).
    """
    nc = bacc.Bacc("TRN2", target_bir_lowering=False, debug=False, num_devices=NCORES)
    xts_d = nc.declare_dram_parameter("xts", [F, RPC], F32, isOutput=False)
    wc_d = nc.declare_dram_parameter("wc", [F, 2], F32, isOutput=False)
    bb_d = nc.declare_dram_parameter("bb", [P, 1], F32, isOutput=False)
    rtag = None
    if reps > 1 or timing:
        rdim = RDIM[rep_scope]
        rtag = nc.declare_dram_parameter("rtag", [rdim, reps], F32, isOutput=False)
    if timing:
        out = nc.dram_tensor("out_scratch", [RPC, N], F32)
        ok = nc.declare_dram_parameter("ok", [1, 4], F32, isOutput=True)
    else:
        out = nc.declare_dram_parameter("out", [RPC, N], F32, isOutput=True)
        ok = None

    QUEUES = ["sync", "scalar", "gpsimd"]

    with tile.TileContext(nc) as tc:
        with (
            tc.tile_pool(name="consts", bufs=1) as consts,
            tc.tile_pool(name="xsp", bufs=1) as xsp,
            tc.tile_pool(name="s2sp", bufs=1) as s2sp,
            tc.tile_pool(name="s2rep", bufs=2) as s2rep_pool,
            tc.tile_pool(name="tmp", bufs=1) as tmp,
            tc.tile_pool(name="stage", bufs=3) as stagep,
            tc.tile_pool(name="dstage", bufs=2) as dstagep,
            tc.tile_pool(name="psum2", bufs=2, space="PSUM") as psum2,
            tc.tile_pool(name="psum1", bufs=2, space="PSUM") as psum1,
            tc.tile_pool(name="dram", bufs=1, space="DRAM") as dram,
        ):

            def qeng(i):
                return getattr(nc, QUEUES[QPLAN[i % len(QPLAN)]])

            if rtag is not None:
                rtag_sb = consts.tile(list(rtag.shape), F32, tag="rtag")
                nc.scalar.dma_start(out=rtag_sb, in_=rtag[:, :])

            def load_consts():
                wc_sb = consts.tile([P, 2, 2], F32, tag="wc")
                nc.scalar.dma_start(out=wc_sb[:, 0, :], in_=wc_d[0:P, :])
                nc.scalar.dma_start(out=wc_sb[:, 1, :], in_=wc_d[P : 2 * P, :])
                b_sb = consts.tile([P, 1], F32, tag="bb")
                nc.scalar.dma_start(out=b_sb, in_=bb_d[:, :])
                ones_sb = consts.tile([1, P], BF16, tag="ones")
                nc.vector.memset(ones_sb, 1.0)
                return wc_sb, b_sb, ones_sb

            def load_xts():
                xts_sb = xsp.tile([P, 2, RPC], F32, tag="xts")
                h = RPC // 2
                nc.sync.dma_start(out=xts_sb[:, 0, 0:h], in_=xts_d[0:P, 0:h])
                nc.sync.dma_start(out=xts_sb[:, 0, h:RPC], in_=xts_d[0:P, h:RPC])
                nc.scalar.dma_start(out=xts_sb[:, 1, 0:h], in_=xts_d[P : 2 * P, 0:h])
                nc.scalar.dma_start(
                    out=xts_sb[:, 1, h:RPC], in_=xts_d[P : 2 * P, h:RPC]
                )
                return xts_sb

            def compute_s2s(xts_sb, wc_sb):
                # own 1250 elements of s2 = x @ w2 as a [1, RPC] bf16 row
                s2s_sb = s2sp.tile([1, RPC], BF16, tag="s2s")
                for sj in range(0, RPC, CJ):
                    cw = min(CJ, RPC - sj)
                    pss = psum2.tile([1, CJ], F32, tag="pss")
                    nc.tensor.matmul(
                        out=pss[0:1, :cw],
                        lhsT=wc_sb[:, 0, 1:2],
                        rhs=xts_sb[:, 0, sj : sj + cw],
                        start=True,
                        stop=False,
                    )
                    nc.tensor.matmul(
                        out=pss[0:1, :cw],
                        lhsT=wc_sb[:, 1, 1:2],
                        rhs=xts_sb[:, 1, sj : sj + cw],
                        start=False,
                        stop=True,
                    )
                    nc.vector.tensor_copy(
                        out=s2s_sb[0:1, sj : sj + cw], in_=pss[0:1, :cw]
                    )
                return s2s_sb

            def allgather(s2s_sb):
                # 2.5KB/rank bf16 in, 20KB out
                in_b = dram.tile([1, RPC], BF16, tag="in_b")
                out_b = dram.tile([1, N], BF16, tag="out_b")
                nc.scalar.dma_start(out=in_b[:, :], in_=s2s_sb[:, :])
                nc.gpsimd.collective_compute(
                    "AllGather",
                    mybir.AluOpType.bypass,
                    replica_groups=[list(range(NCORES))],
                    ins=[in_b[:, :]],
                    outs=[out_b[:, :]],
                )
                return out_b

            def compute_s1(xts_sb, wc_sb, b_sb):
                # s1[r0+p] + b -> s1_sb[p, g]
                s1_sb = consts.tile([P, NG], F32, tag="s1")
                for g, (r0, rt) in enumerate(GROUPS):
                    ps1 = psum1.tile([P, 8], F32, tag="ps1")
                    nc.tensor.matmul(
                        out=ps1[:rt, 0:1],
                        lhsT=xts_sb[:, 0, r0 : r0 + rt],
                        rhs=wc_sb[:, 0, 0:1],
                        start=True,
                        stop=False,
                    )
                    nc.tensor.matmul(
                        out=ps1[:rt, 0:1],
                        lhsT=xts_sb[:, 1, r0 : r0 + rt],
                        rhs=wc_sb[:, 1, 0:1],
                        start=False,
                        stop=True,
                    )
                    nc.vector.tensor_scalar_add(
                        out=s1_sb[:rt, g : g + 1],
                        in0=ps1[:rt, 0:1],
                        scalar1=b_sb[:rt, :],
                    )
                return s1_sb

            def replicate(out_b, ones_sb):
                # readback the gathered s2 row, then broadcast it across all
                # 128 partitions with K=1 ones-matmuls on the (idle) PE
                s2row = s2sp.tile([1, N], BF16, tag="s2row")
                nc.sync.dma_start(out=s2row[:, :], in_=out_b[:, :])
                s2_rep = s2rep_pool.tile([P, N], BF16, tag="s2rep")
                for sj in range(0, N, CJ):
                    cw = min(CJ, N - sj)
                    psr = psum2.tile([P, CJ], F32, tag="psr")
                    nc.tensor.matmul(
                        out=psr[:, :cw],
                        lhsT=ones_sb,
                        rhs=s2row[0:1, sj : sj + cw],
                        start=True,
                        stop=True,
                    )
                    nc.vector.tensor_copy(
                        out=s2_rep[:, sj : sj + cw], in_=psr[:, :cw]
                    )
                return s2_rep

            def act_group(s2_rep, s1_sb, g, rt, o_t):
                nc.scalar.activation(
                    out=o_t[:rt, :],
                    in_=s2_rep[:rt, :],
                    func=mybir.ActivationFunctionType.Sigmoid,
                    bias=s1_sb[:rt, g : g + 1],
                    scale=1.0,
                )

            def dve_chunk(s2_rep, s1_sb, g, rt, d_t, c0, cw):
                x_t = tmp.tile([P, CH], BF16, tag="xt")
                t_t = tmp.tile([P, CH], BF16, tag="tt")
                v_t = tmp.tile([P, CH], BF16, tag="vt")
                x, t, v = x_t[:rt, :cw], t_t[:rt, :cw], v_t[:rt, :cw]
                nc.vector.tensor_scalar_add(
                    out=x, in0=s2_rep[:rt, c0 : c0 + cw], scalar1=s1_sb[:rt, g : g + 1]
                )
                nc.vector.tensor_tensor(out=t, in0=x, in1=x, op=mybir.AluOpType.mult)
                nc.vector.tensor_scalar(
                    out=v, in0=t, scalar1=Q5, scalar2=Q3,
                    op0=mybir.AluOpType.mult, op1=mybir.AluOpType.add,
                )
                nc.vector.tensor_tensor(out=v, in0=v, in1=t, op=mybir.AluOpType.mult)
                nc.vector.tensor_scalar_add(out=v, in0=v, scalar1=Q1)
                nc.vector.tensor_tensor(out=x, in0=x, in1=v, op=mybir.AluOpType.mult)
                nc.vector.tensor_scalar(
                    out=x, in0=x, scalar1=0.5, scalar2=0.0,
                    op0=mybir.AluOpType.add, op1=mybir.AluOpType.max,
                )
                nc.vector.tensor_scalar_min(out=d_t[:rt, :cw], in0=x, scalar1=1.0)

            def out_group_dma(g, r0, rt, o_t, di):
                # full groups: one 5MB DMA; tail: split into 2 halves
                if rt == P:
                    qeng(di).dma_start(out=out[r0 : r0 + rt, :], in_=o_t[:rt, :])
                    return di + 1
                for c0 in (0, N // 2):
                    qeng(di).dma_start(
                        out=out[r0 : r0 + rt, c0 : c0 + N // 2],
                        in_=o_t[:rt, c0 : c0 + N // 2],
                    )
                    di += 1
                return di

            # ---------------- probe scopes ----------------
            if rep_scope in ("dmaA", "dmaD", "dmaE", "dmaF", "dmaG", "dmaH"):
                # big-DMA rate probes, [128, N] tiles, rows 128g:
                # A: round-robin sync/scalar/gpsimd   D: sync/scalar
                # E: gpsimd only   F: sync only   G: gpsimd/sync
                # H: gpsimd-heavy (g,s,g,a)
                qmap = {
                    "dmaA": [0, 1, 2],
                    "dmaD": [0, 1],
                    "dmaE": [2],
                    "dmaF": [0],
                    "dmaG": [2, 0],
                    "dmaH": [2, 0, 2, 1],
                }[rep_scope]
                tiles = []
                for i in range(3):
                    o_t = stagep.tile([P, N], F32, tag="o_t", name=f"st{i}")
                    nc.vector.memset(o_t, 0.25)
                    tiles.append(o_t)
                di = 0
                for _r in range(reps):
                    for g in range(9):
                        r0 = g * P
                        e = getattr(nc, QUEUES[qmap[di % len(qmap)]])
                        e.dma_start(out=out[r0 : r0 + P, :], in_=tiles[g % 3][:, :])
                        di += 1
            elif rep_scope == "coll":
                wc_sb, b_sb, ones_sb = load_consts()
                xts_sb = load_xts()
                s2s_sb = compute_s2s(xts_sb, wc_sb)
                for _r in range(reps):
                    out_b = allgather(s2s_sb)
                s2_rep = replicate(out_b, ones_sb)
                o_t = stagep.tile([P, N], F32, tag="o_t")
                s1_sb = compute_s1(xts_sb, wc_sb, b_sb)
                act_group(s2_rep, s1_sb, 0, P, o_t)
                nc.sync.dma_start(out=out[0:P, :], in_=o_t[:, :])
            elif rep_scope == "repl":
                wc_sb, b_sb, ones_sb = load_consts()
                xts_sb = load_xts()
                s2s_sb = compute_s2s(xts_sb, wc_sb)
                out_b = allgather(s2s_sb)
                s1_sb = compute_s1(xts_sb, wc_sb, b_sb)
                for _r in range(reps):
                    s2_rep = replicate(out_b, ones_sb)
                o_t = stagep.tile([P, N], F32, tag="o_t")
                act_group(s2_rep, s1_sb, 0, P, o_t)
                nc.sync.dma_start(out=out[0:P, :], in_=o_t[:, :])
            elif rep_scope in ("act10k", "dve", "prod", "all"):
                per_rep = rep_scope == "all"
                wc_sb, b_sb, ones_sb = load_consts()
                if not per_rep:
                    xts_sb = load_xts()
                    s2s_sb = compute_s2s(xts_sb, wc_sb)
                    out_b = allgather(s2s_sb)
                    s1_sb = compute_s1(xts_sb, wc_sb, b_sb)
                    s2_rep = replicate(out_b, ones_sb)
                for _r in range(reps):
                    if per_rep:
                        xts_sb = load_xts()
                        s2s_sb = compute_s2s(xts_sb, wc_sb)
                        out_b = allgather(s2s_sb)
                        s1_sb = compute_s1(xts_sb, wc_sb, b_sb)
                        s2_rep = replicate(out_b, ones_sb)
                    if rep_scope == "act10k":
                        for g, (r0, rt) in enumerate(GROUPS):
                            o_t = stagep.tile([P, N], F32, tag="o_t")
                            act_group(s2_rep, s1_sb, g, rt, o_t)
                        continue
                    if rep_scope == "dve":
                        for c0 in range(0, N, CH):
                            cw = min(CH, N - c0)
                            d_t = dstagep.tile([P, CH], F32, tag="d_t")
                            dve_chunk(s2_rep, s1_sb, 8, P, d_t, c0, cw)
                        continue
                    do_dma = rep_scope == "all"
                    di = 0
                    for g, (r0, rt) in enumerate(GROUPS):
                        # DVE takes the last NDVE full (128-row) groups
                        if NDVE > 0 and 9 - NDVE <= g < 9:
                            for c0 in range(0, N, CH):
                                cw = min(CH, N - c0)
                                d_t = dstagep.tile([P, CH], F32, tag="d_t")
                                dve_chunk(s2_rep, s1_sb, g, rt, d_t, c0, cw)
                                if do_dma:
                                    qeng(di).dma_start(
                                        out=out[r0 : r0 + rt, c0 : c0 + cw],
                                        in_=d_t[:rt, :cw],
                                    )
                                    di += 1
                        else:
                            o_t = stagep.tile([P, N], F32, tag="o_t")
                            act_group(s2_rep, s1_sb, g, rt, o_t)
                            if do_dma:
                                di = out_group_dma(g, r0, rt, o_t, di)

            if ok is not None:
                okt = consts.tile([1, 4], F32, tag="okt")
                nc.sync.dma_start(out=okt, in_=out[0:1, 0:4])
                nc.sync.dma_start(out=ok[:, :], in_=okt)
    nc.compile()
    return nc


_NC = {}


def _get_nc(reps=1, timing=False, rep_scope="all"):
    key = (reps, timing, rep_scope)
    if key not in _NC:
        _NC[key] = build_bass(reps=reps, timing=timing, rep_scope=rep_scope)
    return _NC[key]


def make_in_maps(x, w, b):
    xT = np.ascontiguousarray(x.T)  # (F, N)
    wc = np.ascontiguousarray(np.stack([w[0, :F], w[0, F:]], axis=1))  # (F, 2)
    bb = np.full((P, 1), np.float32(b[0]), dtype=np.float32)
    in_maps = []
    for c in range(NCORES):
        xts = np.ascontiguousarray(xT[:, c * RPC : (c + 1) * RPC])
        in_maps.append({"xts": xts, "wc": wc, "bb": bb})
    return in_maps


def kernel(x, adj, w, b):
    x = np.asarray(x, dtype=np.float32)
    w = np.asarray(w, dtype=np.float32)
    b = np.asarray(b, dtype=np.float32)
    nc = _get_nc()
    in_maps = make_in_maps(x, w, b)
    res = run_bass_kernel_spmd(nc, in_maps, list(range(NCORES)))
    return np.concatenate([res.results[c]["out"] for c in range(NCORES)], axis=0)


# revision 16
# speedup vs baseline: 2.6797x; 1.1118x over previous
"""Trainium2 Bass kernel for nn_DenseAtt: att[i,j] = sigmoid(x[i]@w1 + x[j]@w2 + b).

Sharding: rows of the (N, N) output split across 8 NeuronCores (1250 rows
each, as 10 groups of 128 rows; the last group overlaps the 9th by 30 rows
so every DMA keeps 128 partitions — sub-128-partition DMAs run ~2.4x
slower).  Per core:
  1. load its (F, 1250) x^T slab; compute its 1250 elements of s2 = x@w2
     on the PE (cast bf16) and AllGather them (20KB) across the 8 cores,
  2. compute s1 = x_slab@w1 + b as per-partition columns [128, 10],
  3. replicate the gathered s2 row into a [128, N] bf16 SBUF tile with
     K=1 ones-matmuls on the otherwise-idle PE (bf16 keeps SBUF small;
     its rounding adds <2e-3 error vs the 2e-2 tolerance),
  4. ACT streams sigmoid(s2 + s1[p]) per group (exact LUT, one full-width
     [128, 10000] instruction per group; ~1 elem/cycle/lane ~= 670 GB/s),
  5. write the slab with 5MB full-row-contiguous DMAs alternating over the
     two HWDGE queues (sync/scalar), which drain concurrently; a single
     queue or a 3-way mix with gpsimd both measured slower.
Optionally (NDVE>0) DVE produces the last full group(s) with a clamped
minimax quintic sigmoid in bf16 to offload ACT.
"""

import numpy as np

import concourse.bacc as bacc
import concourse.tile as tile
from concourse import mybir
from concourse.bass_utils import run_bass_kernel_spmd

N = 10000
F = 256
NCORES = 8
RPC = N // NCORES  # rows per core = 1250
P = 128
CJ = 512  # PSUM f32 bank chunk
CH = 2000  # DVE column chunk
NDVE = 0  # full groups produced by DVE instead of ACT

# row groups per core: 10 x 128, the last overlapping the 9th by 30 rows
# (rows 1122-1151 are written twice with identical values).  Sub-128-
# partition DMAs run ~2.4x slower, so a full overlapped group wins.
GROUPS = [(g * P, P) for g in range(9)] + [(RPC - P, P)]
NG = len(GROUPS)

# minimax clamped quintic for sigmoid on [-6.9, 6.9] (max err 9.5e-3):
# sigmoid(x) ~= clip(0.5 + x*(Q1 + t*(Q3 + Q5*t)), 0, 1), t = x*x
Q1 = 0.23343955
Q3 = -0.01114885
Q5 = 0.00024648

F32 = mybir.dt.float32
BF16 = mybir.dt.bfloat16

# output-DMA queue plan: queue index (0=sync,1=scalar,2=gpsimd) per DMA op,
# cycled.  Two concurrent queues saturate the HBM write path; mixing all
# three measured slower.  Overridable for experiments via env QPLAN="02".
import os as _os

QPLAN = [int(c) for c in _os.environ.get("QPLAN", "01")]

RDIM = {
    "all": 1,
    "coll": 2,
    "repl": 3,
    "act10k": 4,
    "dve": 5,
    "prod": 6,
    "dmaA": 11,
    "dmaD": 14,
    "dmaE": 15,
    "dmaF": 16,
    "dmaG": 17,
    "dmaH": 18,
}


def build_bass(reps=1, timing=False, rep_scope="all"):
    """Per-core SPMD program.  Inputs (per core):
    xts (F, RPC) : x^T slab of this core's rows (f-major)
    wc  (F, 2)   : [w1 | w2] as columns
    bb  (P, 1)   : bias replicated per partition
    out (RPC, N) : this core's output slab

    reps/timing/rep_scope: differential-timing variants (see test.py/probe.py).
    """
    nc = bacc.Bacc("TRN2", target_bir_lowering=False, debug=False, num_devices=NCORES)
    xts_d = nc.declare_dram_parameter("xts", [F, RPC], F32, isOutput=False)
    wc_d = nc.declare_dram_parameter("wc", [F, 2], F32, isOutput=False)
    bb_d = nc.declare_dram_parameter("bb", [P, 1], F32, isOutput=False)
    rtag = None
    if reps > 1 or timing:
        rdim = RDIM[rep_scope]
        rtag = nc.declare_dram_parameter("rtag", [rdim, reps], F32, isOutput=False)
    if timing:
        out = nc.dram_tensor("out_scratch", [RPC, N], F32)
        ok = nc.declare_dram_parameter("ok", [1, 4], F32, isOutput=True)
    else:
        out = nc.declare_dram_parameter("out", [RPC, N], F32, isOutput=True)
        ok = None

    QUEUES = ["sync", "scalar", "gpsimd"]

    with tile.TileContext(nc) as tc:
        with (
            tc.tile_pool(name="consts", bufs=1) as consts,
            tc.tile_pool(name="xsp", bufs=1) as xsp,
            tc.tile_pool(name="s2sp", bufs=1) as s2sp,
            tc.tile_pool(name="s2rep", bufs=2) as s2rep_pool,
            tc.tile_pool(name="tmp", bufs=1) as tmp,
            tc.tile_pool(name="stage", bufs=3) as stagep,
            tc.tile_pool(name="dstage", bufs=2) as dstagep,
            tc.tile_pool(name="psum2", bufs=2, space="PSUM") as psum2,
            tc.tile_pool(name="psum1", bufs=2, space="PSUM") as psum1,
            tc.tile_pool(name="dram", bufs=1, space="DRAM") as dram,
        ):

            def qeng(i):
                return getattr(nc, QUEUES[QPLAN[i % len(QPLAN)]])

            if rtag is not None:
                rtag_sb = consts.tile(list(rtag.shape), F32, tag="rtag")
                nc.scalar.dma_start(out=rtag_sb, in_=rtag[:, :])

            def load_consts():
                # all prep DMAs ride the gpsimd queue: the output stream owns
                # the sync/scalar queues, and queues are FIFO per engine --
                # prep for rep k+1 must not wait behind rep k's 50MB drain
                wc_sb = consts.tile([P, 2, 2], F32, tag="wc")
                nc.gpsimd.dma_start(out=wc_sb[:, 0, :], in_=wc_d[0:P, :])
                nc.gpsimd.dma_start(out=wc_sb[:, 1, :], in_=wc_d[P : 2 * P, :])
                b_sb = consts.tile([P, 1], F32, tag="bb")
                nc.gpsimd.dma_start(out=b_sb, in_=bb_d[:, :])
                ones_sb = consts.tile([1, P], BF16, tag="ones")
                nc.vector.memset(ones_sb, 1.0)
                return wc_sb, b_sb, ones_sb

            def load_xts():
                xts_sb = xsp.tile([P, 2, RPC], F32, tag="xts")
                h = RPC // 2
                nc.gpsimd.dma_start(out=xts_sb[:, 0, 0:h], in_=xts_d[0:P, 0:h])
                nc.gpsimd.dma_start(out=xts_sb[:, 0, h:RPC], in_=xts_d[0:P, h:RPC])
                nc.gpsimd.dma_start(out=xts_sb[:, 1, 0:h], in_=xts_d[P : 2 * P, 0:h])
                nc.gpsimd.dma_start(
                    out=xts_sb[:, 1, h:RPC], in_=xts_d[P : 2 * P, h:RPC]
                )
                return xts_sb

            def compute_s2s(xts_sb, wc_sb):
                # own 1250 elements of s2 = x @ w2 as a [1, RPC] bf16 row
                s2s_sb = s2sp.tile([1, RPC], BF16, tag="s2s")
                for sj in range(0, RPC, CJ):
                    cw = min(CJ, RPC - sj)
                    pss = psum2.tile([1, CJ], F32, tag="pss")
                    nc.tensor.matmul(
                        out=pss[0:1, :cw],
                        lhsT=wc_sb[:, 0, 1:2],
                        rhs=xts_sb[:, 0, sj : sj + cw],
                        start=True,
                        stop=False,
                    )
                    nc.tensor.matmul(
                        out=pss[0:1, :cw],
                        lhsT=wc_sb[:, 1, 1:2],
                        rhs=xts_sb[:, 1, sj : sj + cw],
                        start=False,
                        stop=True,
                    )
                    nc.vector.tensor_copy(
                        out=s2s_sb[0:1, sj : sj + cw], in_=pss[0:1, :cw]
                    )
                return s2s_sb

            def allgather(s2s_sb):
                # 2.5KB/rank bf16 in, 20KB out
                in_b = dram.tile([1, RPC], BF16, tag="in_b")
                out_b = dram.tile([1, N], BF16, tag="out_b")
                nc.gpsimd.dma_start(out=in_b[:, :], in_=s2s_sb[:, :])
                nc.gpsimd.collective_compute(
                    "AllGather",
                    mybir.AluOpType.bypass,
                    replica_groups=[list(range(NCORES))],
                    ins=[in_b[:, :]],
                    outs=[out_b[:, :]],
                )
                return out_b

            def compute_s1(xts_sb, wc_sb, b_sb):
                # s1[r0+p] + b -> s1_sb[p, g]
                s1_sb = consts.tile([P, NG], F32, tag="s1")
                for g, (r0, rt) in enumerate(GROUPS):
                    ps1 = psum1.tile([P, 8], F32, tag="ps1")
                    nc.tensor.matmul(
                        out=ps1[:rt, 0:1],
                        lhsT=xts_sb[:, 0, r0 : r0 + rt],
                        rhs=wc_sb[:, 0, 0:1],
                        start=True,
                        stop=False,
                    )
                    nc.tensor.matmul(
                        out=ps1[:rt, 0:1],
                        lhsT=xts_sb[:, 1, r0 : r0 + rt],
                        rhs=wc_sb[:, 1, 0:1],
                        start=False,
                        stop=True,
                    )
                    nc.vector.tensor_scalar_add(
                        out=s1_sb[:rt, g : g + 1],
                        in0=ps1[:rt, 0:1],
                        scalar1=b_sb[:rt, :],
                    )
                return s1_sb

            def replicate(out_b, ones_sb):
                # readback the gathered s2 row, then broadcast it across all
                # 128 partitions with K=1 ones-matmuls on the (idle) PE
                s2row = s2sp.tile([1, N], BF16, tag="s2row")
                nc.gpsimd.dma_start(out=s2row[:, :], in_=out_b[:, :])
                s2_rep = s2rep_pool.tile([P, N], BF16, tag="s2rep")
                for sj in range(0, N, CJ):
                    cw = min(CJ, N - sj)
                    psr = psum2.tile([P, CJ], F32, tag="psr")
                    nc.tensor.matmul(
                        out=psr[:, :cw],
                        lhsT=ones_sb,
                        rhs=s2row[0:1, sj : sj + cw],
                        start=True,
                        stop=True,
                    )
                    nc.vector.tensor_copy(
                        out=s2_rep[:, sj : sj + cw], in_=psr[:, :cw]
                    )
                return s2_rep

            def act_group(s2_rep, s1_sb, g, rt, o_t):
                nc.scalar.activation(
                    out=o_t[:rt, :],
                    in_=s2_rep[:rt, :],
                    func=mybir.ActivationFunctionType.Sigmoid,
                    bias=s1_sb[:rt, g : g + 1],
                    scale=1.0,
                )

            def dve_chunk(s2_rep, s1_sb, g, rt, d_t, c0, cw):
                x_t = tmp.tile([P, CH], BF16, tag="xt")
                t_t = tmp.tile([P, CH], BF16, tag="tt")
                v_t = tmp.tile([P, CH], BF16, tag="vt")
                x, t, v = x_t[:rt, :cw], t_t[:rt, :cw], v_t[:rt, :cw]
                nc.vector.tensor_scalar_add(
                    out=x, in0=s2_rep[:rt, c0 : c0 + cw], scalar1=s1_sb[:rt, g : g + 1]
                )
                nc.vector.tensor_tensor(out=t, in0=x, in1=x, op=mybir.AluOpType.mult)
                nc.vector.tensor_scalar(
                    out=v, in0=t, scalar1=Q5, scalar2=Q3,
                    op0=mybir.AluOpType.mult, op1=mybir.AluOpType.add,
                )
                nc.vector.tensor_tensor(out=v, in0=v, in1=t, op=mybir.AluOpType.mult)
                nc.vector.tensor_scalar_add(out=v, in0=v, scalar1=Q1)
                nc.vector.tensor_tensor(out=x, in0=x, in1=v, op=mybir.AluOpType.mult)
                nc.vector.tensor_scalar(
                    out=x, in0=x, scalar1=0.5, scalar2=0.0,
                    op0=mybir.AluOpType.add, op1=mybir.AluOpType.max,
                )
                nc.vector.tensor_scalar_min(out=d_t[:rt, :cw], in0=x, scalar1=1.0)

            def out_group_dma(g, r0, rt, o_t, di):
                # full groups: one 5MB DMA; tail: split into 2 halves
                if rt == P:
                    qeng(di).dma_start(out=out[r0 : r0 + rt, :], in_=o_t[:rt, :])
                    return di + 1
                for c0 in (0, N // 2):
                    qeng(di).dma_start(
                        out=out[r0 : r0 + rt, c0 : c0 + N // 2],
                        in_=o_t[:rt, c0 : c0 + N // 2],
                    )
                    di += 1
                return di

            # ---------------- probe scopes ----------------
            if rep_scope in ("dmaA", "dmaD", "dmaE", "dmaF", "dmaG", "dmaH"):
                # big-DMA rate probes, [128, N] tiles, rows 128g:
                # A: round-robin sync/scalar/gpsimd   D: sync/scalar
                # E: gpsimd only   F: sync only   G: gpsimd/sync
                # H: gpsimd-heavy (g,s,g,a)
                qmap = {
                    "dmaA": [0, 1, 2],
                    "dmaD": [0, 1],
                    "dmaE": [2],
                    "dmaF": [0],
                    "dmaG": [2, 0],
                    "dmaH": [2, 0, 2, 1],
                }[rep_scope]
                tiles = []
                for i in range(3):
                    o_t = stagep.tile([P, N], F32, tag="o_t", name=f"st{i}")
                    nc.vector.memset(o_t, 0.25)
                    tiles.append(o_t)
                di = 0
                for _r in range(reps):
                    for g in range(9):
                        r0 = g * P
                        e = getattr(nc, QUEUES[qmap[di % len(qmap)]])
                        e.dma_start(out=out[r0 : r0 + P, :], in_=tiles[g % 3][:, :])
                        di += 1
            elif rep_scope == "coll":
                wc_sb, b_sb, ones_sb = load_consts()
                xts_sb = load_xts()
                s2s_sb = compute_s2s(xts_sb, wc_sb)
                for _r in range(reps):
                    out_b = allgather(s2s_sb)
                s2_rep = replicate(out_b, ones_sb)
                o_t = stagep.tile([P, N], F32, tag="o_t")
                s1_sb = compute_s1(xts_sb, wc_sb, b_sb)
                act_group(s2_rep, s1_sb, 0, P, o_t)
                nc.sync.dma_start(out=out[0:P, :], in_=o_t[:, :])
            elif rep_scope == "repl":
                wc_sb, b_sb, ones_sb = load_consts()
                xts_sb = load_xts()
                s2s_sb = compute_s2s(xts_sb, wc_sb)
                out_b = allgather(s2s_sb)
                s1_sb = compute_s1(xts_sb, wc_sb, b_sb)
                for _r in range(reps):
                    s2_rep = replicate(out_b, ones_sb)
                o_t = stagep.tile([P, N], F32, tag="o_t")
                act_group(s2_rep, s1_sb, 0, P, o_t)
                nc.sync.dma_start(out=out[0:P, :], in_=o_t[:, :])
            elif rep_scope in ("act10k", "dve", "prod", "all"):
                per_rep = rep_scope == "all"
                wc_sb, b_sb, ones_sb = load_consts()
                if not per_rep:
                    xts_sb = load_xts()
                    s2s_sb = compute_s2s(xts_sb, wc_sb)
                    out_b = allgather(s2s_sb)
                    s1_sb = compute_s1(xts_sb, wc_sb, b_sb)
                    s2_rep = replicate(out_b, ones_sb)
                for _r in range(reps):
                    if per_rep:
                        xts_sb = load_xts()
                        s2s_sb = compute_s2s(xts_sb, wc_sb)
                        out_b = allgather(s2s_sb)
                        s1_sb = compute_s1(xts_sb, wc_sb, b_sb)
                        s2_rep = replicate(out_b, ones_sb)
                    if rep_scope == "act10k":
                        for g, (r0, rt) in enumerate(GROUPS):
                            o_t = stagep.tile([P, N], F32, tag="o_t")
                            act_group(s2_rep, s1_sb, g, rt, o_t)
                        continue
                    if rep_scope == "dve":
                        for c0 in range(0, N, CH):
                            cw = min(CH, N - c0)
                            d_t = dstagep.tile([P, CH], F32, tag="d_t")
                            dve_chunk(s2_rep, s1_sb, 8, P, d_t, c0, cw)
                        continue
                    do_dma = rep_scope == "all"
                    di = 0
                    for g, (r0, rt) in enumerate(GROUPS):
                        # DVE takes the last NDVE full (128-row) groups
                        if NDVE > 0 and 9 - NDVE <= g < 9:
                            for c0 in range(0, N, CH):
                                cw = min(CH, N - c0)
                                d_t = dstagep.tile([P, CH], F32, tag="d_t")
                                dve_chunk(s2_rep, s1_sb, g, rt, d_t, c0, cw)
                                if do_dma:
                                    qeng(di).dma_start(
                                        out=out[r0 : r0 + rt, c0 : c0 + cw],
                                        in_=d_t[:rt, :cw],
                                    )
                                    di += 1
                        else:
                            o_t = stagep.tile([P, N], F32, tag="o_t")
                            act_group(s2_rep, s1_sb, g, rt, o_t)
                            if do_dma:
                                di = out_group_dma(g, r0, rt, o_t, di)

            if ok is not None:
                okt = consts.tile([1, 4], F32, tag="okt")
                nc.sync.dma_start(out=okt, in_=out[0:1, 0:4])
                nc.sync.dma_start(out=ok[:, :], in_=okt)
    nc.compile()
    return nc


_NC = {}


def _get_nc(reps=1, timing=False, rep_scope="all"):
    key = (reps, timing, rep_scope)
    if key not in _NC:
        _NC[key] = build_bass(reps=reps, timing=timing, rep_scope=rep_scope)
    return _NC[key]


def make_in_maps(x, w, b):
    xT = np.ascontiguousarray(x.T)  # (F, N)
    wc = np.ascontiguousarray(np.stack([w[0, :F], w[0, F:]], axis=1))  # (F, 2)
    bb = np.full((P, 1), np.float32(b[0]), dtype=np.float32)
    in_maps = []
    for c in range(NCORES):
        xts = np.ascontiguousarray(xT[:, c * RPC : (c + 1) * RPC])
        in_maps.append({"xts": xts, "wc": wc, "bb": bb})
    return in_maps


def kernel(x, adj, w, b):
    x = np.asarray(x, dtype=np.float32)
    w = np.asarray(w, dtype=np.float32)
    b = np.asarray(b, dtype=np.float32)
    nc = _get_nc()
    in_maps = make_in_maps(x, w, b)
    res = run_bass_kernel_spmd(nc, in_maps, list(range(NCORES)))
    return np.concatenate([res.results[c]["out"] for c in range(NCORES)], axis=0)


# revision 17
# speedup vs baseline: 4.7690x; 1.7796x over previous
"""Trainium2 Bass kernel for nn_DenseAtt: att[i,j] = sigmoid(x[i]@w1 + x[j]@w2 + b).

Sharding: rows of the (N, N) output split across 8 NeuronCores (1250 rows
each, as 10 groups of 128 rows; the last group overlaps the 9th by 30 rows
so every DMA keeps 128 partitions — sub-128-partition DMAs run ~2.4x
slower).  Per core:
  1. load its (F, 1250) x^T slab; compute its 1250 elements of s2 = x@w2
     on the PE (cast bf16) and AllGather them (20KB) across the 8 cores,
  2. compute s1 = x_slab@w1 + b as per-partition columns [128, 10],
  3. replicate the gathered s2 row into a [128, N] bf16 SBUF tile with
     K=1 ones-matmuls on the otherwise-idle PE (bf16 keeps SBUF small;
     its rounding adds <2e-3 error vs the 2e-2 tolerance),
  4. ACT streams sigmoid(s2 + s1[p]) per group (exact LUT, one full-width
     [128, 10000] instruction per group; ~1 elem/cycle/lane ~= 670 GB/s),
  5. write the slab with 5MB full-row-contiguous DMAs alternating over the
     two HWDGE queues (sync/scalar), which drain concurrently; a single
     queue or a 3-way mix with gpsimd both measured slower.
Optionally (NDVE>0) DVE produces the last full group(s) with a clamped
minimax quintic sigmoid in bf16 to offload ACT.
"""

import numpy as np

import concourse.bacc as bacc
import concourse.tile as tile
from concourse import mybir
from concourse.bass_utils import run_bass_kernel_spmd

N = 10000
F = 256
NCORES = 8
RPC = N // NCORES  # rows per core = 1250
P = 128
CJ = 512  # PSUM f32 bank chunk
CH = 2000  # DVE column chunk
NDVE = 0  # full groups produced by DVE instead of ACT

# row groups per core: 10 x 128, the last overlapping the 9th by 30 rows
# (rows 1122-1151 are written twice with identical values).  Sub-128-
# partition DMAs run ~2.4x slower, so a full overlapped group wins.
GROUPS = [(g * P, P) for g in range(9)] + [(RPC - P, P)]
NG = len(GROUPS)

# minimax clamped quintic for sigmoid on [-6.9, 6.9] (max err 9.5e-3):
# sigmoid(x) ~= clip(0.5 + x*(Q1 + t*(Q3 + Q5*t)), 0, 1), t = x*x
Q1 = 0.23343955
Q3 = -0.01114885
Q5 = 0.00024648

F32 = mybir.dt.float32
BF16 = mybir.dt.bfloat16

# output-DMA queue plan: queue index (0=sync,1=scalar,2=gpsimd) per DMA op,
# cycled.  Two concurrent queues saturate the HBM write path; mixing all
# three measured slower.  Overridable for experiments via env QPLAN="02".
import os as _os

QPLAN = [int(c) for c in _os.environ.get("QPLAN", "01")]

RDIM = {
    "all": 1,
    "coll": 2,
    "repl": 3,
    "act10k": 4,
    "dve": 5,
    "prod": 6,
    "dmaA": 11,
    "dmaD": 14,
    "dmaE": 15,
    "dmaF": 16,
    "dmaG": 17,
    "dmaH": 18,
}


def build_bass(reps=1, timing=False, rep_scope="all"):
    """Per-core SPMD program.  Inputs (per core):
    xts (F, RPC) : x^T slab of this core's rows (f-major)
    wc  (F, 2)   : [w1 | w2] as columns
    bb  (P, 1)   : bias replicated per partition
    out (RPC, N) : this core's output slab

    reps/timing/rep_scope: differential-timing variants (see test.py/probe.py).
    """
    nc = bacc.Bacc("TRN2", target_bir_lowering=False, debug=False, num_devices=NCORES)
    xts_d = nc.declare_dram_parameter("xts", [F, RPC], F32, isOutput=False)
    wc_d = nc.declare_dram_parameter("wc", [F, 2], F32, isOutput=False)
    bb_d = nc.declare_dram_parameter("bb", [P, 1], F32, isOutput=False)
    rtag = None
    if reps > 1 or timing:
        rdim = RDIM[rep_scope]
        rtag = nc.declare_dram_parameter("rtag", [rdim, reps], F32, isOutput=False)
    if timing:
        out = nc.dram_tensor("out_scratch", [RPC, N], BF16)
        ok = nc.declare_dram_parameter("ok", [1, 4], BF16, isOutput=True)
    else:
        out = nc.declare_dram_parameter("out", [RPC, N], BF16, isOutput=True)
        ok = None

    QUEUES = ["sync", "scalar", "gpsimd"]

    with tile.TileContext(nc) as tc:
        with (
            tc.tile_pool(name="consts", bufs=1) as consts,
            tc.tile_pool(name="xsp", bufs=1) as xsp,
            tc.tile_pool(name="s2sp", bufs=1) as s2sp,
            tc.tile_pool(name="s2rep", bufs=2) as s2rep_pool,
            tc.tile_pool(name="tmp", bufs=1) as tmp,
            tc.tile_pool(name="stage", bufs=4) as stagep,
            tc.tile_pool(name="dstage", bufs=2) as dstagep,
            tc.tile_pool(name="psum2", bufs=2, space="PSUM") as psum2,
            tc.tile_pool(name="psum1", bufs=2, space="PSUM") as psum1,
            tc.tile_pool(name="dram", bufs=1, space="DRAM") as dram,
        ):

            def qeng(i):
                return getattr(nc, QUEUES[QPLAN[i % len(QPLAN)]])

            if rtag is not None:
                rtag_sb = consts.tile(list(rtag.shape), F32, tag="rtag")
                nc.scalar.dma_start(out=rtag_sb, in_=rtag[:, :])

            def load_consts():
                # all prep DMAs ride the gpsimd queue: the output stream owns
                # the sync/scalar queues, and queues are FIFO per engine --
                # prep for rep k+1 must not wait behind rep k's 50MB drain
                wc_sb = consts.tile([P, 2, 2], F32, tag="wc")
                nc.gpsimd.dma_start(out=wc_sb[:, 0, :], in_=wc_d[0:P, :])
                nc.gpsimd.dma_start(out=wc_sb[:, 1, :], in_=wc_d[P : 2 * P, :])
                b_sb = consts.tile([P, 1], F32, tag="bb")
                nc.gpsimd.dma_start(out=b_sb, in_=bb_d[:, :])
                ones_sb = consts.tile([1, P], BF16, tag="ones")
                nc.vector.memset(ones_sb, 1.0)
                return wc_sb, b_sb, ones_sb

            def load_xts():
                xts_sb = xsp.tile([P, 2, RPC], F32, tag="xts")
                h = RPC // 2
                nc.gpsimd.dma_start(out=xts_sb[:, 0, 0:h], in_=xts_d[0:P, 0:h])
                nc.gpsimd.dma_start(out=xts_sb[:, 0, h:RPC], in_=xts_d[0:P, h:RPC])
                nc.gpsimd.dma_start(out=xts_sb[:, 1, 0:h], in_=xts_d[P : 2 * P, 0:h])
                nc.gpsimd.dma_start(
                    out=xts_sb[:, 1, h:RPC], in_=xts_d[P : 2 * P, h:RPC]
                )
                return xts_sb

            def compute_s2s(xts_sb, wc_sb):
                # own 1250 elements of s2 = x @ w2 as a [1, RPC] bf16 row
                s2s_sb = s2sp.tile([1, RPC], BF16, tag="s2s")
                for sj in range(0, RPC, CJ):
                    cw = min(CJ, RPC - sj)
                    pss = psum2.tile([1, CJ], F32, tag="pss")
                    nc.tensor.matmul(
                        out=pss[0:1, :cw],
                        lhsT=wc_sb[:, 0, 1:2],
                        rhs=xts_sb[:, 0, sj : sj + cw],
                        start=True,
                        stop=False,
                    )
                    nc.tensor.matmul(
                        out=pss[0:1, :cw],
                        lhsT=wc_sb[:, 1, 1:2],
                        rhs=xts_sb[:, 1, sj : sj + cw],
                        start=False,
                        stop=True,
                    )
                    nc.vector.tensor_copy(
                        out=s2s_sb[0:1, sj : sj + cw], in_=pss[0:1, :cw]
                    )
                return s2s_sb

            def allgather(s2s_sb):
                # 2.5KB/rank bf16 in, 20KB out
                in_b = dram.tile([1, RPC], BF16, tag="in_b")
                out_b = dram.tile([1, N], BF16, tag="out_b")
                nc.gpsimd.dma_start(out=in_b[:, :], in_=s2s_sb[:, :])
                nc.gpsimd.collective_compute(
                    "AllGather",
                    mybir.AluOpType.bypass,
                    replica_groups=[list(range(NCORES))],
                    ins=[in_b[:, :]],
                    outs=[out_b[:, :]],
                )
                return out_b

            def compute_s1(xts_sb, wc_sb, b_sb):
                # s1[r0+p] + b -> s1_sb[p, g]
                s1_sb = consts.tile([P, NG], F32, tag="s1")
                for g, (r0, rt) in enumerate(GROUPS):
                    ps1 = psum1.tile([P, 8], F32, tag="ps1")
                    nc.tensor.matmul(
                        out=ps1[:rt, 0:1],
                        lhsT=xts_sb[:, 0, r0 : r0 + rt],
                        rhs=wc_sb[:, 0, 0:1],
                        start=True,
                        stop=False,
                    )
                    nc.tensor.matmul(
                        out=ps1[:rt, 0:1],
                        lhsT=xts_sb[:, 1, r0 : r0 + rt],
                        rhs=wc_sb[:, 1, 0:1],
                        start=False,
                        stop=True,
                    )
                    nc.vector.tensor_scalar_add(
                        out=s1_sb[:rt, g : g + 1],
                        in0=ps1[:rt, 0:1],
                        scalar1=b_sb[:rt, :],
                    )
                return s1_sb

            def replicate(out_b, ones_sb):
                # readback the gathered s2 row, then broadcast it across all
                # 128 partitions with K=1 ones-matmuls on the (idle) PE
                s2row = s2sp.tile([1, N], BF16, tag="s2row")
                nc.gpsimd.dma_start(out=s2row[:, :], in_=out_b[:, :])
                s2_rep = s2rep_pool.tile([P, N], BF16, tag="s2rep")
                for sj in range(0, N, CJ):
                    cw = min(CJ, N - sj)
                    psr = psum2.tile([P, CJ], F32, tag="psr")
                    nc.tensor.matmul(
                        out=psr[:, :cw],
                        lhsT=ones_sb,
                        rhs=s2row[0:1, sj : sj + cw],
                        start=True,
                        stop=True,
                    )
                    nc.vector.tensor_copy(
                        out=s2_rep[:, sj : sj + cw], in_=psr[:, :cw]
                    )
                return s2_rep

            def act_group(s2_rep, s1_sb, g, rt, o_t):
                nc.scalar.activation(
                    out=o_t[:rt, :],
                    in_=s2_rep[:rt, :],
                    func=mybir.ActivationFunctionType.Sigmoid,
                    bias=s1_sb[:rt, g : g + 1],
                    scale=1.0,
                )

            def dve_chunk(s2_rep, s1_sb, g, rt, d_t, c0, cw):
                x_t = tmp.tile([P, CH], BF16, tag="xt")
                t_t = tmp.tile([P, CH], BF16, tag="tt")
                v_t = tmp.tile([P, CH], BF16, tag="vt")
                x, t, v = x_t[:rt, :cw], t_t[:rt, :cw], v_t[:rt, :cw]
                nc.vector.tensor_scalar_add(
                    out=x, in0=s2_rep[:rt, c0 : c0 + cw], scalar1=s1_sb[:rt, g : g + 1]
                )
                nc.vector.tensor_tensor(out=t, in0=x, in1=x, op=mybir.AluOpType.mult)
                nc.vector.tensor_scalar(
                    out=v, in0=t, scalar1=Q5, scalar2=Q3,
                    op0=mybir.AluOpType.mult, op1=mybir.AluOpType.add,
                )
                nc.vector.tensor_tensor(out=v, in0=v, in1=t, op=mybir.AluOpType.mult)
                nc.vector.tensor_scalar_add(out=v, in0=v, scalar1=Q1)
                nc.vector.tensor_tensor(out=x, in0=x, in1=v, op=mybir.AluOpType.mult)
                nc.vector.tensor_scalar(
                    out=x, in0=x, scalar1=0.5, scalar2=0.0,
                    op0=mybir.AluOpType.add, op1=mybir.AluOpType.max,
                )
                nc.vector.tensor_scalar_min(out=d_t[:rt, :cw], in0=x, scalar1=1.0)

            def out_group_dma(g, r0, rt, o_t, di):
                # full groups: one 5MB DMA; tail: split into 2 halves
                if rt == P:
                    qeng(di).dma_start(out=out[r0 : r0 + rt, :], in_=o_t[:rt, :])
                    return di + 1
                for c0 in (0, N // 2):
                    qeng(di).dma_start(
                        out=out[r0 : r0 + rt, c0 : c0 + N // 2],
                        in_=o_t[:rt, c0 : c0 + N // 2],
                    )
                    di += 1
                return di

            # ---------------- probe scopes ----------------
            if rep_scope in ("dmaA", "dmaD", "dmaE", "dmaF", "dmaG", "dmaH"):
                # big-DMA rate probes, [128, N] tiles, rows 128g:
                # A: round-robin sync/scalar/gpsimd   D: sync/scalar
                # E: gpsimd only   F: sync only   G: gpsimd/sync
                # H: gpsimd-heavy (g,s,g,a)
                qmap = {
                    "dmaA": [0, 1, 2],
                    "dmaD": [0, 1],
                    "dmaE": [2],
                    "dmaF": [0],
                    "dmaG": [2, 0],
                    "dmaH": [2, 0, 2, 1],
                }[rep_scope]
                tiles = []
                for i in range(3):
                    o_t = stagep.tile([P, N], BF16, tag="o_t", name=f"st{i}")
                    nc.vector.memset(o_t, 0.25)
                    tiles.append(o_t)
                di = 0
                for _r in range(reps):
                    for g in range(9):
                        r0 = g * P
                        e = getattr(nc, QUEUES[qmap[di % len(qmap)]])
                        e.dma_start(out=out[r0 : r0 + P, :], in_=tiles[g % 3][:, :])
                        di += 1
            elif rep_scope == "coll":
                wc_sb, b_sb, ones_sb = load_consts()
                xts_sb = load_xts()
                s2s_sb = compute_s2s(xts_sb, wc_sb)
                for _r in range(reps):
                    out_b = allgather(s2s_sb)
                s2_rep = replicate(out_b, ones_sb)
                o_t = stagep.tile([P, N], BF16, tag="o_t")
                s1_sb = compute_s1(xts_sb, wc_sb, b_sb)
                act_group(s2_rep, s1_sb, 0, P, o_t)
                nc.sync.dma_start(out=out[0:P, :], in_=o_t[:, :])
            elif rep_scope == "repl":
                wc_sb, b_sb, ones_sb = load_consts()
                xts_sb = load_xts()
                s2s_sb = compute_s2s(xts_sb, wc_sb)
                out_b = allgather(s2s_sb)
                s1_sb = compute_s1(xts_sb, wc_sb, b_sb)
                for _r in range(reps):
                    s2_rep = replicate(out_b, ones_sb)
                o_t = stagep.tile([P, N], BF16, tag="o_t")
                act_group(s2_rep, s1_sb, 0, P, o_t)
                nc.sync.dma_start(out=out[0:P, :], in_=o_t[:, :])
            elif rep_scope in ("act10k", "dve", "prod", "all"):
                per_rep = rep_scope == "all"
                wc_sb, b_sb, ones_sb = load_consts()
                if not per_rep:
                    xts_sb = load_xts()
                    s2s_sb = compute_s2s(xts_sb, wc_sb)
                    out_b = allgather(s2s_sb)
                    s1_sb = compute_s1(xts_sb, wc_sb, b_sb)
                    s2_rep = replicate(out_b, ones_sb)
                for _r in range(reps):
                    if per_rep:
                        xts_sb = load_xts()
                        s2s_sb = compute_s2s(xts_sb, wc_sb)
                        out_b = allgather(s2s_sb)
                        s1_sb = compute_s1(xts_sb, wc_sb, b_sb)
                        s2_rep = replicate(out_b, ones_sb)
                    if rep_scope == "act10k":
                        for g, (r0, rt) in enumerate(GROUPS):
                            o_t = stagep.tile([P, N], BF16, tag="o_t")
                            act_group(s2_rep, s1_sb, g, rt, o_t)
                        continue
                    if rep_scope == "dve":
                        for c0 in range(0, N, CH):
                            cw = min(CH, N - c0)
                            d_t = dstagep.tile([P, CH], BF16, tag="d_t")
                            dve_chunk(s2_rep, s1_sb, 8, P, d_t, c0, cw)
                        continue
                    do_dma = rep_scope == "all"
                    di = 0
                    for g, (r0, rt) in enumerate(GROUPS):
                        # DVE takes the last NDVE full (128-row) groups
                        if NDVE > 0 and 9 - NDVE <= g < 9:
                            for c0 in range(0, N, CH):
                                cw = min(CH, N - c0)
                                d_t = dstagep.tile([P, CH], BF16, tag="d_t")
                                dve_chunk(s2_rep, s1_sb, g, rt, d_t, c0, cw)
                                if do_dma:
                                    qeng(di).dma_start(
                                        out=out[r0 : r0 + rt, c0 : c0 + cw],
                                        in_=d_t[:rt, :cw],
                                    )
                                    di += 1
                        else:
                            o_t = stagep.tile([P, N], BF16, tag="o_t")
                            act_group(s2_rep, s1_sb, g, rt, o_t)
                            if do_dma:
                                di = out_group_dma(g, r0, rt, o_t, di)

            if ok is not None:
                okt = consts.tile([1, 4], BF16, tag="okt")
                nc.sync.dma_start(out=okt, in_=out[0:1, 0:4])
                nc.sync.dma_start(out=ok[:, :], in_=okt)
    nc.compile()
    return nc


_NC = {}


def _get_nc(reps=1, timing=False, rep_scope="all"):
    key = (reps, timing, rep_scope)
    if key not in _NC:
        _NC[key] = build_bass(reps=reps, timing=timing, rep_scope=rep_scope)
    return _NC[key]


def make_in_maps(x, w, b):
    xT = np.ascontiguousarray(x.T)  # (F, N)
    wc = np.ascontiguousarray(np.stack([w[0, :F], w[0, F:]], axis=1))  # (F, 2)
    bb = np.full((P, 1), np.float32(b[0]), dtype=np.float32)
    in_maps = []
    for c in range(NCORES):
        xts = np.ascontiguousarray(xT[:, c * RPC : (c + 1) * RPC])
        in_maps.append({"xts": xts, "wc": wc, "bb": bb})
    return in_maps


def kernel(x, adj, w, b):
    x = np.asarray(x, dtype=np.float32)
    w = np.asarray(w, dtype=np.float32)
    b = np.asarray(b, dtype=np.float32)
    nc = _get_nc()
    in_maps = make_in_maps(x, w, b)
    res = run_bass_kernel_spmd(nc, in_maps, list(range(NCORES)))
    out16 = np.concatenate([res.results[c]["out"] for c in range(NCORES)], axis=0)
    return out16.astype(np.float32)


# revision 18
# speedup vs baseline: 5.0024x; 1.0490x over previous
"""Trainium2 Bass kernel for nn_DenseAtt: att[i,j] = sigmoid(x[i]@w1 + x[j]@w2 + b).

Sharding: rows of the (N, N) output split across 8 NeuronCores (1250 rows
each, as 10 groups of 128 rows; the last group overlaps the 9th by 30 rows
so every DMA keeps 128 partitions — sub-128-partition DMAs run ~2.4x
slower).  Per core:
  1. load its (F, 1250) x^T slab; compute its 1250 elements of s2 = x@w2
     on the PE (cast bf16) and AllGather them (20KB) across the 8 cores,
  2. compute s1 = x_slab@w1 + b as per-partition columns [128, 10],
  3. replicate the gathered s2 row into a [128, N] bf16 SBUF tile with
     K=1 ones-matmuls on the otherwise-idle PE (bf16 keeps SBUF small;
     its rounding adds <2e-3 error vs the 2e-2 tolerance),
  4. ACT streams sigmoid(s2 + s1[p]) per group (exact LUT, one full-width
     [128, 10000] instruction per group; ~1 elem/cycle/lane ~= 670 GB/s),
  5. write the slab with 5MB full-row-contiguous DMAs alternating over the
     two HWDGE queues (sync/scalar), which drain concurrently; a single
     queue or a 3-way mix with gpsimd both measured slower.
Optionally (NDVE>0) DVE produces the last full group(s) with a clamped
minimax quintic sigmoid in bf16 to offload ACT.
"""

import numpy as np

import concourse.bacc as bacc
import concourse.tile as tile
from concourse import mybir
from concourse.bass_utils import run_bass_kernel_spmd

N = 10000
F = 256
NCORES = 8
RPC = N // NCORES  # rows per core = 1250
P = 128
CJ = 512  # PSUM f32 bank chunk
CH = 2000  # DVE column chunk

# row groups per core: 10 x 128, the last overlapping the 9th by 30 rows
# (rows 1122-1151 are written twice with identical values).  Sub-128-
# partition DMAs run ~2.4x slower, so a full overlapped group wins.
GROUPS = [(g * P, P) for g in range(9)] + [(RPC - P, P)]
NG = len(GROUPS)

# minimax clamped quintic for sigmoid on [-6.9, 6.9] (max err 9.5e-3):
# sigmoid(x) ~= clip(0.5 + x*(Q1 + t*(Q3 + Q5*t)), 0, 1), t = x*x
Q1 = 0.23343955
Q3 = -0.01114885
Q5 = 0.00024648

F32 = mybir.dt.float32
BF16 = mybir.dt.bfloat16

# output-DMA queue plan: queue index (0=sync,1=scalar,2=gpsimd) per DMA op,
# cycled.  Two concurrent queues saturate the HBM write path; mixing all
# three measured slower.  Overridable for experiments via env QPLAN="02".
import os as _os

QPLAN = [int(c) for c in _os.environ.get("QPLAN", "01")]
# full groups produced by DVE (quintic approx) instead of ACT, to overlap
# production across both engines; 0 = exact-LUT-only
NDVE = int(_os.environ.get("NDVE", "0"))

RDIM = {
    "all": 1,
    "coll": 2,
    "repl": 3,
    "act10k": 4,
    "dve": 5,
    "prod": 6,
    "dmaA": 11,
    "dmaD": 14,
    "dmaE": 15,
    "dmaF": 16,
    "dmaG": 17,
    "dmaH": 18,
}


def build_bass(reps=1, timing=False, rep_scope="all"):
    """Per-core SPMD program.  Inputs (per core):
    xts (F, RPC) : x^T slab of this core's rows (f-major)
    wc  (F, 2)   : [w1 | w2] as columns
    bb  (P, 1)   : bias replicated per partition
    out (RPC, N) : this core's output slab

    reps/timing/rep_scope: differential-timing variants (see test.py/probe.py).
    """
    nc = bacc.Bacc("TRN2", target_bir_lowering=False, debug=False, num_devices=NCORES)
    xts_d = nc.declare_dram_parameter("xts", [F, RPC], F32, isOutput=False)
    wc_d = nc.declare_dram_parameter("wc", [F, 2], F32, isOutput=False)
    bb_d = nc.declare_dram_parameter("bb", [P, 1], F32, isOutput=False)
    rtag = None
    if reps > 1 or timing:
        rdim = RDIM[rep_scope]
        rtag = nc.declare_dram_parameter("rtag", [rdim, reps], F32, isOutput=False)
    if timing:
        out = nc.dram_tensor("out_scratch", [RPC, N], BF16)
        ok = nc.declare_dram_parameter("ok", [1, 4], BF16, isOutput=True)
    else:
        out = nc.declare_dram_parameter("out", [RPC, N], BF16, isOutput=True)
        ok = None

    QUEUES = ["sync", "scalar", "gpsimd"]

    with tile.TileContext(nc) as tc:
        with (
            tc.tile_pool(name="consts", bufs=1) as consts,
            tc.tile_pool(name="xsp", bufs=1) as xsp,
            tc.tile_pool(name="s2sp", bufs=1) as s2sp,
            tc.tile_pool(name="s2rep", bufs=2) as s2rep_pool,
            tc.tile_pool(name="tmp", bufs=1) as tmp,
            tc.tile_pool(name="stage", bufs=4) as stagep,
            tc.tile_pool(name="dstage", bufs=2) as dstagep,
            tc.tile_pool(name="psum2", bufs=2, space="PSUM") as psum2,
            tc.tile_pool(name="psum1", bufs=2, space="PSUM") as psum1,
            tc.tile_pool(name="dram", bufs=1, space="DRAM") as dram,
        ):

            def qeng(i):
                return getattr(nc, QUEUES[QPLAN[i % len(QPLAN)]])

            if rtag is not None:
                rtag_sb = consts.tile(list(rtag.shape), F32, tag="rtag")
                nc.scalar.dma_start(out=rtag_sb, in_=rtag[:, :])

            def load_consts():
                # all prep DMAs ride the gpsimd queue: the output stream owns
                # the sync/scalar queues, and queues are FIFO per engine --
                # prep for rep k+1 must not wait behind rep k's 50MB drain
                wc_sb = consts.tile([P, 2, 2], F32, tag="wc")
                nc.gpsimd.dma_start(out=wc_sb[:, 0, :], in_=wc_d[0:P, :])
                nc.gpsimd.dma_start(out=wc_sb[:, 1, :], in_=wc_d[P : 2 * P, :])
                b_sb = consts.tile([P, 1], F32, tag="bb")
                nc.gpsimd.dma_start(out=b_sb, in_=bb_d[:, :])
                ones_sb = consts.tile([1, P], BF16, tag="ones")
                nc.vector.memset(ones_sb, 1.0)
                return wc_sb, b_sb, ones_sb

            def load_xts():
                xts_sb = xsp.tile([P, 2, RPC], F32, tag="xts")
                h = RPC // 2
                nc.gpsimd.dma_start(out=xts_sb[:, 0, 0:h], in_=xts_d[0:P, 0:h])
                nc.gpsimd.dma_start(out=xts_sb[:, 0, h:RPC], in_=xts_d[0:P, h:RPC])
                nc.gpsimd.dma_start(out=xts_sb[:, 1, 0:h], in_=xts_d[P : 2 * P, 0:h])
                nc.gpsimd.dma_start(
                    out=xts_sb[:, 1, h:RPC], in_=xts_d[P : 2 * P, h:RPC]
                )
                return xts_sb

            def compute_s2s(xts_sb, wc_sb):
                # own 1250 elements of s2 = x @ w2 as a [1, RPC] bf16 row
                s2s_sb = s2sp.tile([1, RPC], BF16, tag="s2s")
                for sj in range(0, RPC, CJ):
                    cw = min(CJ, RPC - sj)
                    pss = psum2.tile([1, CJ], F32, tag="pss")
                    nc.tensor.matmul(
                        out=pss[0:1, :cw],
                        lhsT=wc_sb[:, 0, 1:2],
                        rhs=xts_sb[:, 0, sj : sj + cw],
                        start=True,
                        stop=False,
                    )
                    nc.tensor.matmul(
                        out=pss[0:1, :cw],
                        lhsT=wc_sb[:, 1, 1:2],
                        rhs=xts_sb[:, 1, sj : sj + cw],
                        start=False,
                        stop=True,
                    )
                    nc.vector.tensor_copy(
                        out=s2s_sb[0:1, sj : sj + cw], in_=pss[0:1, :cw]
                    )
                return s2s_sb

            def allgather(s2s_sb):
                # 2.5KB/rank bf16 in, 20KB out
                in_b = dram.tile([1, RPC], BF16, tag="in_b")
                out_b = dram.tile([1, N], BF16, tag="out_b")
                nc.gpsimd.dma_start(out=in_b[:, :], in_=s2s_sb[:, :])
                nc.gpsimd.collective_compute(
                    "AllGather",
                    mybir.AluOpType.bypass,
                    replica_groups=[list(range(NCORES))],
                    ins=[in_b[:, :]],
                    outs=[out_b[:, :]],
                )
                return out_b

            def compute_s1(xts_sb, wc_sb, b_sb):
                # s1[r0+p] + b -> s1_sb[p, g]
                s1_sb = consts.tile([P, NG], F32, tag="s1")
                for g, (r0, rt) in enumerate(GROUPS):
                    ps1 = psum1.tile([P, 8], F32, tag="ps1")
                    nc.tensor.matmul(
                        out=ps1[:rt, 0:1],
                        lhsT=xts_sb[:, 0, r0 : r0 + rt],
                        rhs=wc_sb[:, 0, 0:1],
                        start=True,
                        stop=False,
                    )
                    nc.tensor.matmul(
                        out=ps1[:rt, 0:1],
                        lhsT=xts_sb[:, 1, r0 : r0 + rt],
                        rhs=wc_sb[:, 1, 0:1],
                        start=False,
                        stop=True,
                    )
                    nc.vector.tensor_scalar_add(
                        out=s1_sb[:rt, g : g + 1],
                        in0=ps1[:rt, 0:1],
                        scalar1=b_sb[:rt, :],
                    )
                return s1_sb

            def replicate(out_b, ones_sb):
                # readback the gathered s2 row, then broadcast it across all
                # 128 partitions with K=1 ones-matmuls on the (idle) PE
                s2row = s2sp.tile([1, N], BF16, tag="s2row")
                nc.gpsimd.dma_start(out=s2row[:, :], in_=out_b[:, :])
                s2_rep = s2rep_pool.tile([P, N], BF16, tag="s2rep")
                for sj in range(0, N, CJ):
                    cw = min(CJ, N - sj)
                    psr = psum2.tile([P, CJ], F32, tag="psr")
                    nc.tensor.matmul(
                        out=psr[:, :cw],
                        lhsT=ones_sb,
                        rhs=s2row[0:1, sj : sj + cw],
                        start=True,
                        stop=True,
                    )
                    nc.vector.tensor_copy(
                        out=s2_rep[:, sj : sj + cw], in_=psr[:, :cw]
                    )
                return s2_rep

            def act_group(s2_rep, s1_sb, g, rt, o_t):
                nc.scalar.activation(
                    out=o_t[:rt, :],
                    in_=s2_rep[:rt, :],
                    func=mybir.ActivationFunctionType.Sigmoid,
                    bias=s1_sb[:rt, g : g + 1],
                    scale=1.0,
                )

            def dve_chunk(s2_rep, s1_sb, g, rt, d_t, c0, cw):
                x_t = tmp.tile([P, CH], BF16, tag="xt")
                t_t = tmp.tile([P, CH], BF16, tag="tt")
                v_t = tmp.tile([P, CH], BF16, tag="vt")
                x, t, v = x_t[:rt, :cw], t_t[:rt, :cw], v_t[:rt, :cw]
                nc.vector.tensor_scalar_add(
                    out=x, in0=s2_rep[:rt, c0 : c0 + cw], scalar1=s1_sb[:rt, g : g + 1]
                )
                nc.vector.tensor_tensor(out=t, in0=x, in1=x, op=mybir.AluOpType.mult)
                nc.vector.tensor_scalar(
                    out=v, in0=t, scalar1=Q5, scalar2=Q3,
                    op0=mybir.AluOpType.mult, op1=mybir.AluOpType.add,
                )
                nc.vector.tensor_tensor(out=v, in0=v, in1=t, op=mybir.AluOpType.mult)
                nc.vector.tensor_scalar_add(out=v, in0=v, scalar1=Q1)
                nc.vector.tensor_tensor(out=x, in0=x, in1=v, op=mybir.AluOpType.mult)
                nc.vector.tensor_scalar(
                    out=x, in0=x, scalar1=0.5, scalar2=0.0,
                    op0=mybir.AluOpType.add, op1=mybir.AluOpType.max,
                )
                nc.vector.tensor_scalar_min(
                    out=d_t[:rt, c0 : c0 + cw], in0=x, scalar1=1.0
                )

            def out_group_dma(g, r0, rt, o_t, di):
                # full groups: one 5MB DMA; tail: split into 2 halves
                if rt == P:
                    qeng(di).dma_start(out=out[r0 : r0 + rt, :], in_=o_t[:rt, :])
                    return di + 1
                for c0 in (0, N // 2):
                    qeng(di).dma_start(
                        out=out[r0 : r0 + rt, c0 : c0 + N // 2],
                        in_=o_t[:rt, c0 : c0 + N // 2],
                    )
                    di += 1
                return di

            # ---------------- probe scopes ----------------
            if rep_scope in ("dmaA", "dmaD", "dmaE", "dmaF", "dmaG", "dmaH"):
                # big-DMA rate probes, [128, N] tiles, rows 128g:
                # A: round-robin sync/scalar/gpsimd   D: sync/scalar
                # E: gpsimd only   F: sync only   G: gpsimd/sync
                # H: gpsimd-heavy (g,s,g,a)
                qmap = {
                    "dmaA": [0, 1, 2],
                    "dmaD": [0, 1],
                    "dmaE": [2],
                    "dmaF": [0],
                    "dmaG": [2, 0],
                    "dmaH": [2, 0, 2, 1],
                }[rep_scope]
                tiles = []
                for i in range(3):
                    o_t = stagep.tile([P, N], BF16, tag="o_t", name=f"st{i}")
                    nc.vector.memset(o_t, 0.25)
                    tiles.append(o_t)
                di = 0
                for _r in range(reps):
                    for g in range(9):
                        r0 = g * P
                        e = getattr(nc, QUEUES[qmap[di % len(qmap)]])
                        e.dma_start(out=out[r0 : r0 + P, :], in_=tiles[g % 3][:, :])
                        di += 1
            elif rep_scope == "coll":
                wc_sb, b_sb, ones_sb = load_consts()
                xts_sb = load_xts()
                s2s_sb = compute_s2s(xts_sb, wc_sb)
                for _r in range(reps):
                    out_b = allgather(s2s_sb)
                s2_rep = replicate(out_b, ones_sb)
                o_t = stagep.tile([P, N], BF16, tag="o_t")
                s1_sb = compute_s1(xts_sb, wc_sb, b_sb)
                act_group(s2_rep, s1_sb, 0, P, o_t)
                nc.sync.dma_start(out=out[0:P, :], in_=o_t[:, :])
            elif rep_scope == "repl":
                wc_sb, b_sb, ones_sb = load_consts()
                xts_sb = load_xts()
                s2s_sb = compute_s2s(xts_sb, wc_sb)
                out_b = allgather(s2s_sb)
                s1_sb = compute_s1(xts_sb, wc_sb, b_sb)
                for _r in range(reps):
                    s2_rep = replicate(out_b, ones_sb)
                o_t = stagep.tile([P, N], BF16, tag="o_t")
                act_group(s2_rep, s1_sb, 0, P, o_t)
                nc.sync.dma_start(out=out[0:P, :], in_=o_t[:, :])
            elif rep_scope in ("act10k", "dve", "prod", "all"):
                per_rep = rep_scope == "all"
                wc_sb, b_sb, ones_sb = load_consts()
                if not per_rep:
                    xts_sb = load_xts()
                    s2s_sb = compute_s2s(xts_sb, wc_sb)
                    out_b = allgather(s2s_sb)
                    s1_sb = compute_s1(xts_sb, wc_sb, b_sb)
                    s2_rep = replicate(out_b, ones_sb)
                for _r in range(reps):
                    if per_rep:
                        xts_sb = load_xts()
                        s2s_sb = compute_s2s(xts_sb, wc_sb)
                        out_b = allgather(s2s_sb)
                        s1_sb = compute_s1(xts_sb, wc_sb, b_sb)
                        s2_rep = replicate(out_b, ones_sb)
                    if rep_scope == "act10k":
                        for g, (r0, rt) in enumerate(GROUPS):
                            o_t = stagep.tile([P, N], BF16, tag="o_t")
                            act_group(s2_rep, s1_sb, g, rt, o_t)
                        continue
                    if rep_scope == "dve":
                        d_t = stagep.tile([P, N], BF16, tag="o_t")
                        for c0 in range(0, N, CH):
                            cw = min(CH, N - c0)
                            dve_chunk(s2_rep, s1_sb, 8, P, d_t, c0, cw)
                        continue
                    do_dma = rep_scope == "all"
                    di = 0
                    for g, (r0, rt) in enumerate(GROUPS):
                        # DVE takes the last NDVE full (128-row) groups
                        if NDVE > 0 and 9 - NDVE <= g < 9:
                            o_t = stagep.tile([P, N], BF16, tag="o_t")
                            for c0 in range(0, N, CH):
                                cw = min(CH, N - c0)
                                dve_chunk(s2_rep, s1_sb, g, rt, o_t, c0, cw)
                            if do_dma:
                                di = out_group_dma(g, r0, rt, o_t, di)
                        else:
                            o_t = stagep.tile([P, N], BF16, tag="o_t")
                            act_group(s2_rep, s1_sb, g, rt, o_t)
                            if do_dma:
                                di = out_group_dma(g, r0, rt, o_t, di)

            if ok is not None:
                okt = consts.tile([1, 4], BF16, tag="okt")
                nc.sync.dma_start(out=okt, in_=out[0:1, 0:4])
                nc.sync.dma_start(out=ok[:, :], in_=okt)
    nc.compile()
    return nc


_NC = {}


def _get_nc(reps=1, timing=False, rep_scope="all"):
    key = (reps, timing, rep_scope)
    if key not in _NC:
        _NC[key] = build_bass(reps=reps, timing=timing, rep_scope=rep_scope)
    return _NC[key]


def make_in_maps(x, w, b):
    xT = np.ascontiguousarray(x.T)  # (F, N)
    wc = np.ascontiguousarray(np.stack([w[0, :F], w[0, F:]], axis=1))  # (F, 2)
    bb = np.full((P, 1), np.float32(b[0]), dtype=np.float32)
    in_maps = []
    for c in range(NCORES):
        xts = np.ascontiguousarray(xT[:, c * RPC : (c + 1) * RPC])
        in_maps.append({"xts": xts, "wc": wc, "bb": bb})
    return in_maps


def kernel(x, adj, w, b):
    x = np.asarray(x, dtype=np.float32)
    w = np.asarray(w, dtype=np.float32)
    b = np.asarray(b, dtype=np.float32)
    nc = _get_nc()
    in_maps = make_in_maps(x, w, b)
    res = run_bass_kernel_spmd(nc, in_maps, list(range(NCORES)))
    out16 = np.concatenate([res.results[c]["out"] for c in range(NCORES)], axis=0)
    return out16.astype(np.float32)
